# revision 52
# baseline (speedup 1.0000x reference)
"""GCN layer with virtual node on 8 Trainium2 NeuronCores (Bass/Tile).

Reference computation (fp32):
    agg = segment_sum(H[src], dst, N)        # message passing
    out = H + agg
    vmean = segment_mean(out, batch, G)      # virtual node
    out = out + vmean[batch]
    y = relu(out @ W)

Distribution strategy (self-contained, hardcoded):
  - batch is sorted, G=256 graphs, 8 cores -> graph-aligned node sharding
    (cuts chosen to minimize the max per-core edge count subject to a node
    cap). Per-graph means never cross cores: no collectives needed.
  - Edges are partitioned by owning core of dst, sorted by (group, src
    class, window, src) and packed densely into per-(group, class) regions
    (class = src // 25088, int16 gather reach). One dma_gather per (group,
    class), issued on SWDGE queue = class: the Q7 descriptor generation
    (~6.7 ns/desc on ONE Q7 pair) is the hard bottleneck, and queue q is
    generated by Q7 pair (2q, 2q+1), so 4 balanced classes on 4 queues
    generate in parallel.
  - segment_sum via PE one-hot matmul: slot s carries tag (dst&127) +
    128*(w%16); R[s, m] = (tag[s] == iota), built on DVE in one batched op
    per (window, class) run over the window's static tile range.
  - virtual node: b_all[dst, w, g] = (batch_rel == g), built in ONE DVE op;
    psum_s[g, f] += b_all_w^T @ out_w over all windows; vmean = psum_s *
    (1/cnt); vW = vmean @ W once. bT[g, dst] built directly on DVE from a
    partition-replicated batch vector (no PE transposes).
  - final: psum_y = oT_w @ W + bT_w @ vW accumulated in one PSUM tile;
    relu out of PSUM on the Scalar engine (DVE stays off the critical
    path). PSUM->SBUF copies are quad-batched (4 windows per bank) and run
    on Scalar.
"""
import os
import numpy as np

from concourse import bass, bacc, mybir
import concourse.tile as tile
from concourse.bass_utils import run_bass_kernel_spmd

P = 128
N_CORES = 8
D = 128
F32 = mybir.dt.float32
F16 = mybir.dt.float16
I32 = mybir.dt.int32
I16 = mybir.dt.int16
GROUP_W = 8           # windows per gather group
CLASS_SIZE = 25000    # int16 index reach; 4 exactly-equal classes -> 4 queues
WMOD = 16             # window disambiguation modulus in drel encoding
QW = 4                # windows per PSUM quad


def _ceil(a, b):
    return -(-a // b)


# ---------------------------------------------------------------------------
# host-side prep: pure index arithmetic / sharding metadata
# ---------------------------------------------------------------------------

def _prep(H, edge_index, batch, n_graphs):
    N = H.shape[0]
    src = np.asarray(edge_index[0], dtype=np.int64)
    dst = np.asarray(edge_index[1], dtype=np.int64)
    batch = np.asarray(batch, dtype=np.int64)
    n_cls = _ceil(N, CLASS_SIZE)

    gstart = np.searchsorted(batch, np.arange(n_graphs + 1))
    counts = np.diff(gstart)

    # graph-aligned cuts minimizing the max per-core edge count (the Q7
    # descriptor-generation wall) subject to a per-core node cap that keeps
    # NW at its uniform-cut value.
    epg = np.bincount(batch[dst], minlength=n_graphs).astype(np.int64)
    npg = counts.astype(np.int64)
    ncap = _ceil(int(np.diff(gstart[::n_graphs // N_CORES]).max()), P) * P

    def cuts_for(bound):
        cut = [0]
        ce = cn = 0
        for gi in range(n_graphs):
            if ce + epg[gi] > bound or cn + npg[gi] > ncap:
                if len(cut) == N_CORES:
                    return None
                cut.append(gi)
                ce = cn = 0
            ce += epg[gi]
            cn += npg[gi]
        return cut + [n_graphs] if len(cut) == N_CORES else None

    lo_b, hi_b = int(epg.sum() // N_CORES), int(epg.sum())
    while lo_b < hi_b:
        mid = (lo_b + hi_b) // 2
        if cuts_for(mid) is None:
            lo_b = mid + 1
        else:
            hi_b = mid
    best = cuts_for(lo_b)
    if best is None:  # node cap infeasible for greedy: uniform fallback
        best = list(range(0, n_graphs, n_graphs // N_CORES))[:N_CORES] \
            + [n_graphs]
    gcut = np.asarray(best, dtype=np.int64)
    gpc = int(np.diff(gcut).max())  # max graphs per core (tile dim)
    core_start = gstart[gcut]

    gcore = (np.searchsorted(gcut, batch, side="right") - 1).astype(np.int64)
    ecore = gcore[dst]

    n_c = np.diff(core_start)
    NW = int(_ceil(n_c.max(), P))
    # group sizes: small head groups (fast pipeline ramp), GROUP_W middle,
    # tapered tail (short drain: the last group's scatter work trails the
    # final gather)
    sizes = []
    rem = NW
    for s in (4,):
        if rem > s + 12:
            sizes.append(s)
            rem -= s
    tail = [s for s in (4, 2) if s < rem]
    rem -= sum(tail)
    sizes += [GROUP_W] * (rem // GROUP_W)
    if rem % GROUP_W:
        sizes.append(rem % GROUP_W)
    sizes += tail
    gb = [0]
    for s in sizes:
        gb.append(gb[-1] + s)
    assert gb[-1] == NW
    NGRP = len(gb) - 1
    grpmap = np.searchsorted(np.asarray(gb[1:]), np.arange(NW), side='right')

    percore = []
    cnt_gk = np.zeros((N_CORES, NGRP, n_cls), dtype=np.int64)
    for c in range(N_CORES):
        m = ecore == c
        s_c, d_c = src[m], dst[m]
        dstl = d_c - core_start[c]
        w = dstl >> 7
        k = s_c // CLASS_SIZE
        g = grpmap[w]
        order = np.lexsort((s_c, w, k, g))
        s_c, dstl, w, k, g = (s_c[order], dstl[order], w[order], k[order],
                              g[order])
        gk = g * n_cls + k
        cnt_gk[c] = np.bincount(gk, minlength=NGRP * n_cls).reshape(
            NGRP, n_cls)
        percore.append((s_c, dstl, w, k, g))

    cap_gk = (_ceil(cnt_gk.max(axis=0), P) * P).astype(np.int64)  # [NGRP,ncls]
    capt_gk = cap_gk // P
    rt0 = np.concatenate([[0], np.cumsum(capt_gk.ravel())]).reshape(
        -1)  # region tile starts, flat (g, k) order
    TT = int(rt0[-1])

    lo_wk = np.full((NW, n_cls), 1 << 30, dtype=np.int64)
    hi_wk = np.zeros((NW, n_cls), dtype=np.int64)
    in_maps = []
    for c in range(N_CORES):
        s_c, dstl, w, k, g_arr = percore[c]
        idx_flat = np.full(TT * P, -1, dtype=np.int64)
        drel = np.full((P, TT), -1.0, dtype=np.float16)
        tag = ((dstl & 127) + 128 * (w % WMOD)).astype(np.float16)
        for g in range(NGRP):
            w0, w1 = gb[g], gb[g + 1]
            for kk in range(n_cls):
                base_slot = int(rt0[g * n_cls + kk]) * P
                sel = (g_arr == g) & (k == kk)
                n = int(sel.sum())
                gs = base_slot + np.arange(n)
                idx_flat[gs] = s_c[sel] - CLASS_SIZE * kk
                drel[gs % P, gs // P] = tag[sel]
                if n == 0:
                    idx_flat[base_slot] = 0  # cnt>=1 pad (drel=-1)
                # per-window runs -> static tile ranges
                cnt_w = np.bincount(w[sel] - w0, minlength=w1 - w0)
                off = base_slot + np.concatenate(
                    [[0], np.cumsum(cnt_w)]).astype(np.int64)
                for wi in range(w0, w1):
                    a, b = off[wi - w0], off[wi - w0 + 1]
                    if b > a:
                        lo_wk[wi, kk] = min(lo_wk[wi, kk], a // P)
                        hi_wk[wi, kk] = max(hi_wk[wi, kk], _ceil(b, P))

        # wrap int16 indices into [16, cap/16] blocks per (g, k) region,
        # replicated to 128 partitions
        wrapped = np.zeros((P, TT * P // 16), dtype=np.int16)
        for g in range(NGRP):
            for kk in range(n_cls):
                ccap = int(cap_gk[g, kk])
                if ccap == 0:
                    continue
                sbase = int(rt0[g * n_cls + kk]) * P
                block = idx_flat[sbase:sbase + ccap]
                wb = block.reshape(ccap // 16, 16).T.astype(np.int16)
                col0 = sbase // 16
                wrapped[:16, col0:col0 + ccap // 16] = wb
        wrapped[16:] = np.tile(wrapped[:16], (7, 1))

        nodes = int(n_c[c])
        hcore = np.zeros((NW * P, D), dtype=np.float32)
        hcore[:nodes] = H[core_start[c]:core_start[c] + nodes]
        hcp = np.ascontiguousarray(
            hcore.reshape(NW, P, D).transpose(1, 0, 2)).astype(np.float16)
        brel = np.full((NW * P,), -1.0, dtype=np.float16)
        brel[:nodes] = (batch[core_start[c]:core_start[c] + nodes]
                        - gcut[c]).astype(np.float16)
        brel2 = brel.reshape(NW, P).T.copy()
        brelr = np.ascontiguousarray(
            np.broadcast_to(brel.reshape(1, NW * P), (gpc, NW * P)))
        invc = np.ones((gpc, 1), dtype=np.float32)
        ngr = int(gcut[c + 1] - gcut[c])
        invc[:ngr, 0] = 1.0 / np.maximum(counts[gcut[c]:gcut[c + 1]], 1)

        in_maps.append({
            "idx16": np.ascontiguousarray(wrapped),
            "cnt": np.ascontiguousarray(
                np.maximum(cnt_gk[c], 1).reshape(
                    1, NGRP * n_cls).astype(np.int32)),
            "drel": np.ascontiguousarray(drel),
            "hcp": hcp,
            "brel": np.ascontiguousarray(brel2),
            "brelr": brelr,
            "invc": invc,
        })

    for wi in range(NW):
        for kk in range(n_cls):
            if lo_wk[wi, kk] >= (1 << 30):
                lo_wk[wi, kk] = hi_wk[wi, kk] = 0
    # every window must have at least one nonempty run (else PSUM quad
    # slices would stay uninitialized)
    for wi in range(NW):
        assert any(hi_wk[wi, kk] > lo_wk[wi, kk] for kk in range(n_cls)), wi
    # mod-WMOD safety: same-residue windows must not have overlapping spans
    for kk in range(n_cls):
        for wi in range(NW):
            for wj in range(wi + 1, NW):
                if wi % WMOD == wj % WMOD:
                    if not (hi_wk[wj, kk] <= lo_wk[wi, kk]
                            or lo_wk[wj, kk] >= hi_wk[wi, kk]):
                        raise RuntimeError(
                            f"mod-{WMOD} window collision {wi},{wj},{kk}")

    params = dict(
        N=N, NW=NW, NGRP=NGRP, gpc=gpc, TT=TT, n_cls=n_cls,
        gb=tuple(int(x) for x in gb),
        capt_gk=tuple(int(x) for x in capt_gk.ravel()),
        rt0=tuple(int(x) for x in rt0),
        lo_wk=tuple(int(x) for x in lo_wk.ravel()),
        hi_wk=tuple(int(x) for x in hi_wk.ravel()),
        cls_size=tuple(min(CLASS_SIZE, N - CLASS_SIZE * kk)
                       for kk in range(n_cls)),
    )
    return params, in_maps, n_c, core_start


def _consts(params, W):
    iota1024 = np.broadcast_to(
        np.arange(WMOD * P, dtype=np.float16), (P, WMOD * P)).copy()
    warmix = np.zeros((P, 8), dtype=np.int16)
    iotag = np.broadcast_to(
        np.arange(params["gpc"], dtype=np.float16), (P, params["gpc"])).copy()
    iotagc = np.arange(params["gpc"], dtype=np.float16).reshape(-1, 1).copy()
    ident = np.eye(P, dtype=np.float16)
    return {"iota1024": iota1024, "warmix": warmix,
            "iotag": iotag, "iotagc": iotagc,
            "ident": ident, "w16": np.ascontiguousarray(W, dtype=np.float16)}


# ---------------------------------------------------------------------------
# device kernel builder (SPMD: one program, per-core data)
# ---------------------------------------------------------------------------

def _build(params):
    NW, NGRP, TT = params["NW"], params["NGRP"], params["TT"]
    gpc, n_cls = params["gpc"], params["n_cls"]
    capt_gk = params["capt_gk"]
    gb = params["gb"]
    rt0 = params["rt0"]
    lo_wk, hi_wk = params["lo_wk"], params["hi_wk"]
    cls_size = params["cls_size"]
    N = params["N"]
    NTMAX = int(max(max(hi_wk[i] - lo_wk[i] for i in range(NW * n_cls)), 1))
    GRP_TILES = [sum(capt_gk[g * n_cls + kk] for kk in range(n_cls))
                 for g in range(NGRP)]
    GT_MAX = max(GRP_TILES)

    nc = bacc.Bacc("TRN2", target_bir_lowering=False, debug=False,
                   num_devices=N_CORES, num_swdge_queues=4)
    hfull_d = nc.dram_tensor("hfull16", [N, D], F16, kind="ExternalInput")
    idx_d = nc.dram_tensor("idx16", [P, TT * P // 16], I16,
                           kind="ExternalInput")
    cnt_d = nc.dram_tensor("cnt", [1, NGRP * n_cls], I32,
                           kind="ExternalInput")
    warmix_d = nc.dram_tensor("warmix", [P, 8], I16, kind="ExternalInput")
    drel_d = nc.dram_tensor("drel", [P, TT], F16, kind="ExternalInput")
    hcp_d = nc.dram_tensor("hcp", [P, NW, D], F16, kind="ExternalInput")
    brel_d = nc.dram_tensor("brel", [P, NW], F16, kind="ExternalInput")
    brelr_d = nc.dram_tensor("brelr", [gpc, NW * P], F16,
                             kind="ExternalInput")
    invc_d = nc.dram_tensor("invc", [gpc, 1], F32, kind="ExternalInput")
    iota_d = nc.dram_tensor("iota1024", [P, WMOD * P], F16,
                            kind="ExternalInput")
    iotag_d = nc.dram_tensor("iotag", [P, gpc], F16, kind="ExternalInput")
    iotagc_d = nc.dram_tensor("iotagc", [gpc, 1], F16, kind="ExternalInput")
    ident_d = nc.dram_tensor("ident", [P, P], F16, kind="ExternalInput")
    w16_d = nc.dram_tensor("w16", [P, D], F16, kind="ExternalInput")
    y_d = nc.dram_tensor("y", [NW * P, D], F32, kind="ExternalOutput")

    with tile.TileContext(nc) as tc:
        with tc.tile_pool(name="const", bufs=1) as cpool, \
             nc.gpsimd.register("gcnt") as gcnt:
            cnt_t = cpool.tile([1, NGRP * n_cls], I32)
            nc.sync.dma_start(out=cnt_t[:], in_=cnt_d[:])
            warmix_t = cpool.tile([P, 8], I16)
            nc.sync.dma_start(out=warmix_t[:], in_=warmix_d[:])
            idxg = []
            gspan = []
            for g in range(NGRP):
                c0 = rt0[g * n_cls] * 8
                c1 = rt0[(g + 1) * n_cls] * 8
                gspan.append((c0, c1))
                idxg.append(cpool.tile([P, c1 - c0], I16, name=f"idxg{g}"))
            # group-0 indices first so the first gather starts early; the
            # remaining groups' index loads hide under gen.
            nc.sync.dma_start(out=idxg[0][:], in_=idx_d[:, gspan[0][0]:
                                                        gspan[0][1]])
            iota_t = cpool.tile([P, WMOD * P], F16)
            nc.sync.dma_start(out=iota_t[:], in_=iota_d[:])
            iotag_t = cpool.tile([P, gpc], F16)
            nc.sync.dma_start(out=iotag_t[:], in_=iotag_d[:])
            iotagc_t = cpool.tile([gpc, 1], F16)
            nc.sync.dma_start(out=iotagc_t[:], in_=iotagc_d[:])
            ident_t = cpool.tile([P, P], F16)
            nc.sync.dma_start(out=ident_t[:], in_=ident_d[:])
            w16_t = cpool.tile([P, D], F16)
            nc.sync.dma_start(out=w16_t[:], in_=w16_d[:])
            invc_t = cpool.tile([gpc, 1], F32)
            nc.sync.dma_start(out=invc_t[:], in_=invc_d[:])
            brel_t = cpool.tile([P, NW], F16)
            nc.sync.dma_start(out=brel_t[:], in_=brel_d[:])
            drel_t = cpool.tile([P, TT], F16)
            nc.sync.dma_start(out=drel_t[:], in_=drel_d[:])
            out_sb = cpool.tile([P, NW, D], F16)
            nc.sync.dma_start(out=out_sb[:], in_=hcp_d[:])
            for g in range(1, NGRP):
                nc.sync.dma_start(out=idxg[g][:],
                                  in_=idx_d[:, gspan[g][0]:gspan[g][1]])
            b_all = cpool.tile([P, NW, gpc], F16)
            bT_all = cpool.tile([gpc, NW, P], F16)
            vw_sb = cpool.tile([gpc, D], F16)

            with tc.tile_pool(name="gpool", bufs=3) as gpool, \
                 tc.tile_pool(name="rpool", bufs=4) as rpool, \
                 tc.tile_pool(name="pw", bufs=2, space="PSUM") as pwpool, \
                 tc.tile_pool(name="pt", bufs=2, space="PSUM") as ptpool, \
                 tc.tile_pool(name="ps", bufs=1, space="PSUM") as pspool:

                # G buffers memset FIRST on DVE (no input deps) so the
                # first two groups' gathers are not stalled behind the
                # one-hot builds or group-0's R work. Every byte of both
                # buffers must be finite before first use: static (min/max
                # over cores) tile ranges may read slots this core never
                # gathers.
                g_bufs = [gpool.tile([P, GT_MAX, D], F16, tag="G",
                                     name=f"gbuf{i}")
                          for i in range(3)]
                # warm each queue's Q7 pair (one-time ~6us IRAM library
                # load) with a tiny 128-idx gather into scratch before the
                # real stream begins
                for kk in range(min(n_cls, 4)):
                    nc.gpsimd.dma_gather(
                        out_ap=g_bufs[2][:, kk:kk + 1, :],
                        in_ap=hfull_d[0:cls_size[0], :],
                        idxs_ap=warmix_t[:],
                        num_idxs=P,
                        num_idxs_reg=P,
                        elem_size=D,
                        single_packet=False,
                        queue_num=kk % 4,
                    )
                # first-group memset split per class region so gather (0,k)
                # only waits for its own piece
                gt0 = 0
                for kk in range(n_cls):
                    c = capt_gk[kk]
                    if c:
                        nc.vector.memset(g_bufs[0][:, gt0:gt0 + c, :], 0.0)
                    gt0 += c
                nc.vector.memset(g_bufs[1][:], 0.0)
                nc.vector.memset(g_bufs[2][:], 0.0)
                if gt0 < GT_MAX:
                    nc.vector.memset(g_bufs[0][:, gt0:, :], 0.0)

                # batched one-hot builds (one DVE op each):
                # b_all[p, w, g] = (brel[p, w] == g)
                in0 = bass.AP(brel_t[:].tensor, brel_t[:].offset,
                              [list(brel_t[:].ap[0]), [1, NW], [0, gpc]])
                in1 = bass.AP(iotag_t[:].tensor, iotag_t[:].offset,
                              [list(iotag_t[:].ap[0]), [0, NW], [1, gpc]])
                nc.vector.tensor_tensor(out=b_all[:], in0=in0, in1=in1,
                                        op=mybir.AluOpType.is_equal)
                # bT_all[g, j] = (brel_flat[j] == g), staged in 16-window
                # chunks to fit SBUF next to the G buffers
                CHW = 16
                with tc.tile_pool(name="brp", bufs=1) as brpool:
                    for h, j0 in enumerate(range(0, NW, CHW)):
                        j1 = min(j0 + CHW, NW)
                        cols = (j1 - j0) * P
                        brelr_t = brpool.tile([gpc, CHW * P], F16,
                                              tag="brl", name=f"brl{h}")
                        nc.sync.dma_start(
                            out=brelr_t[:, :cols],
                            in_=brelr_d[:, j0 * P:j1 * P])
                        nc.vector.tensor_tensor(
                            out=bT_all[:, j0:j1, :],
                            in0=brelr_t[:, :cols],
                            in1=iotagc_t[:, 0:1].to_broadcast([gpc, cols]),
                            op=mybir.AluOpType.is_equal)

                psum_s = pspool.tile([gpc, D], F32, space="PSUM")

                for g in range(NGRP):
                    w0, w1 = gb[g], gb[g + 1]
                    gbase = rt0[g * n_cls]  # first tile of this group
                    g_t = g_bufs[g % 3]
                    for kk in range(n_cls):
                        capt = capt_gk[g * n_cls + kk]
                        if capt == 0:
                            continue
                        rbase = rt0[g * n_cls + kk]
                        nc.gpsimd.load(
                            gcnt,
                            cnt_t[0:1, g * n_cls + kk:g * n_cls + kk + 1])
                        base = CLASS_SIZE * kk
                        nc.gpsimd.dma_gather(
                            out_ap=g_t[:, rbase - gbase:rbase - gbase + capt,
                                       :],
                            in_ap=hfull_d[base:base + cls_size[kk], :],
                            idxs_ap=idxg[g][:, (rbase - gbase) * 8:
                                            (rbase - gbase + capt) * 8],
                            num_idxs=capt * P,
                            num_idxs_reg=gcnt,
                            elem_size=D,
                            single_packet=False,
                            queue_num=kk % 4,
                        )

                    for wq in range(w0, w1, QW):
                        q = min(QW, w1 - wq)
                        psum_w = pwpool.tile([P, QW, D], F32, space="PSUM",
                                             tag="pw")
                        for j in range(q):
                            w = wq + j
                            runs = []
                            for kk in range(n_cls):
                                lo, hi = (lo_wk[w * n_cls + kk],
                                          hi_wk[w * n_cls + kk])
                                if hi > lo:
                                    runs.append((lo, hi))
                            for ri, (lo, hi) in enumerate(runs):
                                nt = hi - lo
                                r_t = rpool.tile([P, NTMAX, P], F16, tag="R")
                                in0 = drel_t[:, lo:hi].to_broadcast(
                                    [P, nt, P])
                                sl = iota_t[:, 128 * (w % WMOD):
                                            128 * (w % WMOD) + P]
                                in1 = bass.AP(
                                    sl.tensor, sl.offset,
                                    [list(sl.ap[0]), [0, nt], list(sl.ap[1])])
                                nc.vector.tensor_tensor(
                                    out=r_t[:, :nt, :], in0=in0, in1=in1,
                                    op=mybir.AluOpType.is_equal)
                                last_run = ri == len(runs) - 1
                                for t in range(nt):
                                    nc.tensor.matmul(
                                        psum_w[:, j, :],
                                        r_t[:, t, :],
                                        g_t[:, lo - gbase + t, :],
                                        start=(ri == 0 and t == 0),
                                        stop=(last_run and t == nt - 1),
                                        skip_group_check=True)
                        nc.vector.tensor_tensor(
                            out=out_sb[:, wq:wq + q, :],
                            in0=psum_w[:, :q, :],
                            in1=out_sb[:, wq:wq + q, :],
                            op=mybir.AluOpType.add)
                        psum_oT = ptpool.tile([P, QW, P], F16, space="PSUM",
                                              tag="poT")
                        for j in range(q):
                            w = wq + j
                            nc.tensor.matmul(
                                psum_s[:], b_all[:, w, :], out_sb[:, w, :],
                                start=(w == 0), stop=(w == NW - 1),
                                skip_group_check=True)
                            nc.tensor.transpose(psum_oT[:, j, :],
                                                out_sb[:, w, :], ident_t[:])
                        oT_q = rpool.tile([P, QW, P], F16, tag="oT")
                        nc.scalar.copy(oT_q[:, :q, :], psum_oT[:, :q, :])
                        # y1 = out @ W computed here (PE has slack); out_sb
                        # rows are dead after psum_s + transpose, so y1
                        # overwrites them in f16. The tail then only needs
                        # bT @ vW + identity-add of y1.
                        psum_y1 = ptpool.tile([P, QW, D], F32, space="PSUM",
                                              tag="py1")
                        for j in range(q):
                            nc.tensor.matmul(psum_y1[:, j, :],
                                             oT_q[:, j, :], w16_t[:],
                                             start=True, stop=True,
                                             skip_group_check=True)
                        nc.scalar.copy(out_sb[:, wq:wq + q, :],
                                       psum_y1[:, :q, :])

                vmean16 = cpool.tile([gpc, D], F16)
                nc.scalar.mul(vmean16[:], psum_s[:], invc_t[:, 0:1])

            with tc.tile_pool(name="p3", bufs=4) as p3, \
                 tc.tile_pool(name="pp1", bufs=1, space="PSUM") as pp1, \
                 tc.tile_pool(name="pp3", bufs=4, space="PSUM") as pp3:
                psum_vmT = pp1.tile([P, gpc], F16, space="PSUM", tag="pvmT")
                nc.tensor.transpose(psum_vmT[:], vmean16[:],
                                    ident_t[0:gpc, 0:gpc])
                vmT = p3.tile([P, gpc], F16, tag="vmT")
                nc.scalar.copy(vmT[:], psum_vmT[:])
                psum_vw = pp1.tile([gpc, D], F32, space="PSUM", tag="pvw")
                nc.tensor.matmul(psum_vw[:], vmT[:], w16_t[:],
                                 start=True, stop=True)
                nc.scalar.copy(vw_sb[:], psum_vw[:])

                for qi, w0 in enumerate(range(0, NW, QW)):
                    q = min(QW, NW - w0)
                    psum_yq = pp3.tile([P, QW, D], F32, space="PSUM",
                                       tag="py")
                    for j in range(q):
                        w = w0 + j
                        nc.tensor.matmul(psum_yq[:, j, :], bT_all[:, w, :],
                                         vw_sb[:], start=True, stop=True,
                                         skip_group_check=True)
                    # += y1 (stored in out_sb) on DVE (idle at the tail),
                    # relu on Scalar, DMA alternating both HWDGE queues
                    ys_t = p3.tile([P, QW, D], F32, tag="YS")
                    nc.vector.tensor_tensor(
                        out=ys_t[:, :q, :], in0=psum_yq[:, :q, :],
                        in1=out_sb[:, w0:w0 + q, :],
                        op=mybir.AluOpType.add)
                    yq_t = p3.tile([P, QW, D], F32, tag="Y")
                    nc.scalar.activation(yq_t[:, :q, :], ys_t[:, :q, :],
                                         mybir.ActivationFunctionType.Relu)
                    out_ap = bass.AP(
                        y_d[:].tensor, w0 * P * D,
                        [[D, P], [P * D, q], [1, D]])
                    eng = nc.sync if qi % 2 == 0 else nc.scalar
                    eng.dma_start(out=out_ap, in_=yq_t[:, :q, :])
    _finish_compile(nc)
    return nc


def _finish_compile(nc):
    nc.compile()
    # compile()'s tail passes (library-load insertion for the custom DMA
    # instructions) can reintroduce >1 sync wait per instruction, which the
    # TRN2 ISA rejects. Re-split and re-codegen.
    import bass_rust
    bass_rust.generate_event_semaphores(nc)
    nc.codegen_inst_isa_subclasses()


_BUILD_CACHE = {}


def _build_cached(params):
    key = tuple(sorted((k, v) for k, v in params.items()))
    if key not in _BUILD_CACHE:
        _BUILD_CACHE[key] = _build(params)
    return _BUILD_CACHE[key]


def _run(H, edge_index, batch, W, n_graphs, trace=False):
    H = np.asarray(H)
    params, in_maps, n_c, core_start = _prep(H, edge_index, batch, n_graphs)
    consts = _consts(params, np.asarray(W))
    hfull16 = np.ascontiguousarray(H.astype(np.float16))
    for m in in_maps:
        m["hfull16"] = hfull16
        m.update(consts)
    nc = _build_cached(params)
    res = run_bass_kernel_spmd(nc, in_maps, list(range(N_CORES)), trace=trace)
    N = H.shape[0]
    y = np.empty((N, D), dtype=np.float32)
    for c in range(N_CORES):
        y[core_start[c]:core_start[c] + n_c[c]] = \
            res.results[c]["y"][:n_c[c]]
    return y, res


def kernel(H, edge_index, batch, W):
    y, _ = _run(H, edge_index, batch, W, n_graphs=256,
                trace=bool(os.environ.get("GCN_TRACE")))
    return y


# revision 53
# speedup vs baseline: 1.0234x; 1.0234x over previous
"""GCN layer with virtual node on 8 Trainium2 NeuronCores (Bass/Tile).

Reference computation (fp32):
    agg = segment_sum(H[src], dst, N)        # message passing
    out = H + agg
    vmean = segment_mean(out, batch, G)      # virtual node
    out = out + vmean[batch]
    y = relu(out @ W)

Distribution strategy (self-contained, hardcoded):
  - batch is sorted, G=256 graphs, 8 cores -> graph-aligned node sharding
    (cuts chosen to minimize the max per-core edge count subject to a node
    cap). Per-graph means never cross cores: no collectives needed.
  - Edges are partitioned by owning core of dst, sorted by (group, src
    class, window, src) and packed densely into per-(group, class) regions
    (class = src // 25000, int16 gather reach; 4 exactly-equal classes).
    One dma_gather per (group, class) on SWDGE queue = class: Q7
    descriptor generation (~8 ns/desc per queue) is the hard wall, and
    queue q is generated by Q7 pair (2q, 2q+1), so the 4 classes generate
    on all 8 Q7 cores in parallel (the single-queue baseline used 2).
    Tiny warm-up gathers absorb the one-time Q7 IRAM library load; G
    buffers are triple-buffered so the gather stream never stalls on
    consumers (gathers run back-to-back, ~0 idle).
  - segment_sum via PE one-hot matmul: slot s carries tag (dst&127) +
    128*(w%16); R[s, m] = (tag[s] == iota), built on DVE in one batched op
    per (window, class) run over the window's static tile range (min/max
    over cores; the mod-16 tag zeroes foreign-window slots in shared or
    drifted tiles).
  - virtual node: b_all[dst, w, g] = (batch_rel == g) built in ONE DVE op;
    psum_s[g, f] += b_all_w^T @ out_w over all windows; vmean = psum_s *
    (1/cnt) on Scalar; vW = vmean @ W once. bT[g, dst] built directly on
    DVE from a partition-replicated batch vector (no PE transposes).
  - y1 = out @ W is computed inside the window loop (PE slack) via a
    PE transpose of out_w; y1 overwrites the dead out_sb rows in f16.
    Final stage per window: psum = bT_w @ vW, DVE adds y1, Scalar relus,
    output DMA alternates both HWDGE queues. PSUM->SBUF copies are
    quad-batched (4 windows per PSUM bank) and run on Scalar.
"""
import os
import numpy as np

from concourse import bass, bacc, mybir
import concourse.tile as tile
from concourse.bass_utils import run_bass_kernel_spmd

P = 128
N_CORES = 8
D = 128
F32 = mybir.dt.float32
F16 = mybir.dt.float16
I32 = mybir.dt.int32
I16 = mybir.dt.int16
GROUP_W = 8           # windows per gather group
CLASS_SIZE = 25000    # int16 index reach; 4 exactly-equal classes -> 4 queues
WMOD = 16             # window disambiguation modulus in drel encoding
QW = 4                # windows per PSUM quad


def _ceil(a, b):
    return -(-a // b)


# ---------------------------------------------------------------------------
# host-side prep: pure index arithmetic / sharding metadata
# ---------------------------------------------------------------------------

def _prep(H, edge_index, batch, n_graphs):
    N = H.shape[0]
    src = np.asarray(edge_index[0], dtype=np.int64)
    dst = np.asarray(edge_index[1], dtype=np.int64)
    batch = np.asarray(batch, dtype=np.int64)
    n_cls = _ceil(N, CLASS_SIZE)

    gstart = np.searchsorted(batch, np.arange(n_graphs + 1))
    counts = np.diff(gstart)

    # graph-aligned cuts minimizing the max per-core edge count (the Q7
    # descriptor-generation wall) subject to a per-core node cap that keeps
    # NW at its uniform-cut value.
    epg = np.bincount(batch[dst], minlength=n_graphs).astype(np.int64)
    npg = counts.astype(np.int64)
    ncap = _ceil(int(np.diff(gstart[::n_graphs // N_CORES]).max()), P) * P

    def cuts_for(bound):
        cut = [0]
        ce = cn = 0
        for gi in range(n_graphs):
            if ce + epg[gi] > bound or cn + npg[gi] > ncap:
                if len(cut) == N_CORES:
                    return None
                cut.append(gi)
                ce = cn = 0
            ce += epg[gi]
            cn += npg[gi]
        return cut + [n_graphs] if len(cut) == N_CORES else None

    lo_b, hi_b = int(epg.sum() // N_CORES), int(epg.sum())
    while lo_b < hi_b:
        mid = (lo_b + hi_b) // 2
        if cuts_for(mid) is None:
            lo_b = mid + 1
        else:
            hi_b = mid
    best = cuts_for(lo_b)
    if best is None:  # node cap infeasible for greedy: uniform fallback
        best = list(range(0, n_graphs, n_graphs // N_CORES))[:N_CORES] \
            + [n_graphs]
    gcut = np.asarray(best, dtype=np.int64)
    gpc = int(np.diff(gcut).max())  # max graphs per core (tile dim)
    core_start = gstart[gcut]

    gcore = (np.searchsorted(gcut, batch, side="right") - 1).astype(np.int64)
    ecore = gcore[dst]

    n_c = np.diff(core_start)
    NW = int(_ceil(n_c.max(), P))
    # group sizes: small head groups (fast pipeline ramp), GROUP_W middle,
    # tapered tail (short drain: the last group's scatter work trails the
    # final gather)
    sizes = []
    rem = NW
    for s in (4,):
        if rem > s + 12:
            sizes.append(s)
            rem -= s
    tail = [s for s in (4, 2) if s < rem]
    rem -= sum(tail)
    sizes += [GROUP_W] * (rem // GROUP_W)
    if rem % GROUP_W:
        sizes.append(rem % GROUP_W)
    sizes += tail
    gb = [0]
    for s in sizes:
        gb.append(gb[-1] + s)
    assert gb[-1] == NW
    NGRP = len(gb) - 1
    grpmap = np.searchsorted(np.asarray(gb[1:]), np.arange(NW), side='right')

    percore = []
    cnt_gk = np.zeros((N_CORES, NGRP, n_cls), dtype=np.int64)
    for c in range(N_CORES):
        m = ecore == c
        s_c, d_c = src[m], dst[m]
        dstl = d_c - core_start[c]
        w = dstl >> 7
        k = s_c // CLASS_SIZE
        g = grpmap[w]
        order = np.lexsort((s_c, w, k, g))
        s_c, dstl, w, k, g = (s_c[order], dstl[order], w[order], k[order],
                              g[order])
        gk = g * n_cls + k
        cnt_gk[c] = np.bincount(gk, minlength=NGRP * n_cls).reshape(
            NGRP, n_cls)
        percore.append((s_c, dstl, w, k, g))

    cap_gk = (_ceil(cnt_gk.max(axis=0), P) * P).astype(np.int64)  # [NGRP,ncls]
    capt_gk = cap_gk // P
    rt0 = np.concatenate([[0], np.cumsum(capt_gk.ravel())]).reshape(
        -1)  # region tile starts, flat (g, k) order
    TT = int(rt0[-1])

    lo_wk = np.full((NW, n_cls), 1 << 30, dtype=np.int64)
    hi_wk = np.zeros((NW, n_cls), dtype=np.int64)
    in_maps = []
    for c in range(N_CORES):
        s_c, dstl, w, k, g_arr = percore[c]
        idx_flat = np.full(TT * P, -1, dtype=np.int64)
        drel = np.full((P, TT), -1.0, dtype=np.float16)
        tag = ((dstl & 127) + 128 * (w % WMOD)).astype(np.float16)
        for g in range(NGRP):
            w0, w1 = gb[g], gb[g + 1]
            for kk in range(n_cls):
                base_slot = int(rt0[g * n_cls + kk]) * P
                sel = (g_arr == g) & (k == kk)
                n = int(sel.sum())
                gs = base_slot + np.arange(n)
                idx_flat[gs] = s_c[sel] - CLASS_SIZE * kk
                drel[gs % P, gs // P] = tag[sel]
                if n == 0:
                    idx_flat[base_slot] = 0  # cnt>=1 pad (drel=-1)
                # per-window runs -> static tile ranges
                cnt_w = np.bincount(w[sel] - w0, minlength=w1 - w0)
                off = base_slot + np.concatenate(
                    [[0], np.cumsum(cnt_w)]).astype(np.int64)
                for wi in range(w0, w1):
                    a, b = off[wi - w0], off[wi - w0 + 1]
                    if b > a:
                        lo_wk[wi, kk] = min(lo_wk[wi, kk], a // P)
                        hi_wk[wi, kk] = max(hi_wk[wi, kk], _ceil(b, P))

        # wrap int16 indices into [16, cap/16] blocks per (g, k) region,
        # replicated to 128 partitions
        wrapped = np.zeros((P, TT * P // 16), dtype=np.int16)
        for g in range(NGRP):
            for kk in range(n_cls):
                ccap = int(cap_gk[g, kk])
                if ccap == 0:
                    continue
                sbase = int(rt0[g * n_cls + kk]) * P
                block = idx_flat[sbase:sbase + ccap]
                wb = block.reshape(ccap // 16, 16).T.astype(np.int16)
                col0 = sbase // 16
                wrapped[:16, col0:col0 + ccap // 16] = wb
        wrapped[16:] = np.tile(wrapped[:16], (7, 1))

        nodes = int(n_c[c])
        hcore = np.zeros((NW * P, D), dtype=np.float32)
        hcore[:nodes] = H[core_start[c]:core_start[c] + nodes]
        hcp = np.ascontiguousarray(
            hcore.reshape(NW, P, D).transpose(1, 0, 2)).astype(np.float16)
        brel = np.full((NW * P,), -1.0, dtype=np.float16)
        brel[:nodes] = (batch[core_start[c]:core_start[c] + nodes]
                        - gcut[c]).astype(np.float16)
        brel2 = brel.reshape(NW, P).T.copy()
        brelr = np.ascontiguousarray(
            np.broadcast_to(brel.reshape(1, NW * P), (gpc, NW * P)))
        invc = np.ones((gpc, 1), dtype=np.float32)
        ngr = int(gcut[c + 1] - gcut[c])
        invc[:ngr, 0] = 1.0 / np.maximum(counts[gcut[c]:gcut[c + 1]], 1)

        in_maps.append({
            "idx16": np.ascontiguousarray(wrapped),
            "cnt": np.ascontiguousarray(
                np.maximum(cnt_gk[c], 1).reshape(
                    1, NGRP * n_cls).astype(np.int32)),
            "drel": np.ascontiguousarray(drel),
            "hcp": hcp,
            "brel": np.ascontiguousarray(brel2),
            "brelr": brelr,
            "invc": invc,
        })

    for wi in range(NW):
        for kk in range(n_cls):
            if lo_wk[wi, kk] >= (1 << 30):
                lo_wk[wi, kk] = hi_wk[wi, kk] = 0
    # every window must have at least one nonempty run (else PSUM quad
    # slices would stay uninitialized)
    for wi in range(NW):
        assert any(hi_wk[wi, kk] > lo_wk[wi, kk] for kk in range(n_cls)), wi
    # mod-WMOD safety: same-residue windows must not have overlapping spans
    for kk in range(n_cls):
        for wi in range(NW):
            for wj in range(wi + 1, NW):
                if wi % WMOD == wj % WMOD:
                    if not (hi_wk[wj, kk] <= lo_wk[wi, kk]
                            or lo_wk[wj, kk] >= hi_wk[wi, kk]):
                        raise RuntimeError(
                            f"mod-{WMOD} window collision {wi},{wj},{kk}")

    params = dict(
        N=N, NW=NW, NGRP=NGRP, gpc=gpc, TT=TT, n_cls=n_cls,
        gb=tuple(int(x) for x in gb),
        capt_gk=tuple(int(x) for x in capt_gk.ravel()),
        rt0=tuple(int(x) for x in rt0),
        lo_wk=tuple(int(x) for x in lo_wk.ravel()),
        hi_wk=tuple(int(x) for x in hi_wk.ravel()),
        cls_size=tuple(min(CLASS_SIZE, N - CLASS_SIZE * kk)
                       for kk in range(n_cls)),
    )
    return params, in_maps, n_c, core_start


def _consts(params, W):
    iota1024 = np.broadcast_to(
        np.arange(WMOD * P, dtype=np.float16), (P, WMOD * P)).copy()
    warmix = np.zeros((P, 8), dtype=np.int16)
    iotag = np.broadcast_to(
        np.arange(params["gpc"], dtype=np.float16), (P, params["gpc"])).copy()
    iotagc = np.arange(params["gpc"], dtype=np.float16).reshape(-1, 1).copy()
    ident = np.eye(P, dtype=np.float16)
    return {"iota1024": iota1024, "warmix": warmix,
            "iotag": iotag, "iotagc": iotagc,
            "ident": ident, "w16": np.ascontiguousarray(W, dtype=np.float16)}


# ---------------------------------------------------------------------------
# device kernel builder (SPMD: one program, per-core data)
# ---------------------------------------------------------------------------

def _build(params):
    NW, NGRP, TT = params["NW"], params["NGRP"], params["TT"]
    gpc, n_cls = params["gpc"], params["n_cls"]
    capt_gk = params["capt_gk"]
    gb = params["gb"]
    rt0 = params["rt0"]
    lo_wk, hi_wk = params["lo_wk"], params["hi_wk"]
    cls_size = params["cls_size"]
    N = params["N"]
    NTMAX = int(max(max(hi_wk[i] - lo_wk[i] for i in range(NW * n_cls)), 1))
    GRP_TILES = [sum(capt_gk[g * n_cls + kk] for kk in range(n_cls))
                 for g in range(NGRP)]
    GT_MAX = max(GRP_TILES)

    nc = bacc.Bacc("TRN2", target_bir_lowering=False, debug=False,
                   num_devices=N_CORES, num_swdge_queues=4)
    hfull_d = nc.dram_tensor("hfull16", [N, D], F16, kind="ExternalInput")
    idx_d = nc.dram_tensor("idx16", [P, TT * P // 16], I16,
                           kind="ExternalInput")
    cnt_d = nc.dram_tensor("cnt", [1, NGRP * n_cls], I32,
                           kind="ExternalInput")
    warmix_d = nc.dram_tensor("warmix", [P, 8], I16, kind="ExternalInput")
    drel_d = nc.dram_tensor("drel", [P, TT], F16, kind="ExternalInput")
    hcp_d = nc.dram_tensor("hcp", [P, NW, D], F16, kind="ExternalInput")
    brel_d = nc.dram_tensor("brel", [P, NW], F16, kind="ExternalInput")
    brelr_d = nc.dram_tensor("brelr", [gpc, NW * P], F16,
                             kind="ExternalInput")
    invc_d = nc.dram_tensor("invc", [gpc, 1], F32, kind="ExternalInput")
    iota_d = nc.dram_tensor("iota1024", [P, WMOD * P], F16,
                            kind="ExternalInput")
    iotag_d = nc.dram_tensor("iotag", [P, gpc], F16, kind="ExternalInput")
    iotagc_d = nc.dram_tensor("iotagc", [gpc, 1], F16, kind="ExternalInput")
    ident_d = nc.dram_tensor("ident", [P, P], F16, kind="ExternalInput")
    w16_d = nc.dram_tensor("w16", [P, D], F16, kind="ExternalInput")
    y_d = nc.dram_tensor("y", [NW * P, D], F32, kind="ExternalOutput")

    with tile.TileContext(nc) as tc:
        with tc.tile_pool(name="const", bufs=1) as cpool, \
             nc.gpsimd.register("gcnt") as gcnt:
            cnt_t = cpool.tile([1, NGRP * n_cls], I32)
            nc.sync.dma_start(out=cnt_t[:], in_=cnt_d[:])
            warmix_t = cpool.tile([P, 8], I16)
            nc.sync.dma_start(out=warmix_t[:], in_=warmix_d[:])
            idxg = []
            gspan = []
            for g in range(NGRP):
                c0 = rt0[g * n_cls] * 8
                c1 = rt0[(g + 1) * n_cls] * 8
                gspan.append((c0, c1))
                idxg.append(cpool.tile([P, c1 - c0], I16, name=f"idxg{g}"))
            # group-0 indices first so the first gather starts early; the
            # remaining groups' index loads hide under gen.
            nc.sync.dma_start(out=idxg[0][:], in_=idx_d[:, gspan[0][0]:
                                                        gspan[0][1]])
            iota_t = cpool.tile([P, WMOD * P], F16)
            nc.sync.dma_start(out=iota_t[:], in_=iota_d[:])
            iotag_t = cpool.tile([P, gpc], F16)
            nc.sync.dma_start(out=iotag_t[:], in_=iotag_d[:])
            iotagc_t = cpool.tile([gpc, 1], F16)
            nc.sync.dma_start(out=iotagc_t[:], in_=iotagc_d[:])
            ident_t = cpool.tile([P, P], F16)
            nc.sync.dma_start(out=ident_t[:], in_=ident_d[:])
            w16_t = cpool.tile([P, D], F16)
            nc.sync.dma_start(out=w16_t[:], in_=w16_d[:])
            invc_t = cpool.tile([gpc, 1], F32)
            nc.sync.dma_start(out=invc_t[:], in_=invc_d[:])
            brel_t = cpool.tile([P, NW], F16)
            nc.sync.dma_start(out=brel_t[:], in_=brel_d[:])
            drel_t = cpool.tile([P, TT], F16)
            nc.sync.dma_start(out=drel_t[:], in_=drel_d[:])
            out_sb = cpool.tile([P, NW, D], F16)
            nc.sync.dma_start(out=out_sb[:], in_=hcp_d[:])
            for g in range(1, NGRP):
                nc.sync.dma_start(out=idxg[g][:],
                                  in_=idx_d[:, gspan[g][0]:gspan[g][1]])
            b_all = cpool.tile([P, NW, gpc], F16)
            bT_all = cpool.tile([gpc, NW, P], F16)
            vw_sb = cpool.tile([gpc, D], F16)

            with tc.tile_pool(name="gpool", bufs=3) as gpool, \
                 tc.tile_pool(name="rpool", bufs=4) as rpool, \
                 tc.tile_pool(name="pw", bufs=2, space="PSUM") as pwpool, \
                 tc.tile_pool(name="pt", bufs=2, space="PSUM") as ptpool, \
                 tc.tile_pool(name="ps", bufs=1, space="PSUM") as pspool:

                # G buffers memset FIRST on DVE (no input deps) so the
                # first two groups' gathers are not stalled behind the
                # one-hot builds or group-0's R work. Every byte of both
                # buffers must be finite before first use: static (min/max
                # over cores) tile ranges may read slots this core never
                # gathers.
                g_bufs = [gpool.tile([P, GT_MAX, D], F16, tag="G",
                                     name=f"gbuf{i}")
                          for i in range(3)]
                # warm each queue's Q7 pair (one-time ~6us IRAM library
                # load) with a tiny 128-idx gather into scratch before the
                # real stream begins
                for kk in range(min(n_cls, 4)):
                    nc.gpsimd.dma_gather(
                        out_ap=g_bufs[2][:, kk:kk + 1, :],
                        in_ap=hfull_d[0:cls_size[0], :],
                        idxs_ap=warmix_t[:],
                        num_idxs=P,
                        num_idxs_reg=P,
                        elem_size=D,
                        single_packet=False,
                        queue_num=kk % 4,
                    )
                # first-group memset split per class region so gather (0,k)
                # only waits for its own piece
                gt0 = 0
                for kk in range(n_cls):
                    c = capt_gk[kk]
                    if c:
                        nc.vector.memset(g_bufs[0][:, gt0:gt0 + c, :], 0.0)
                    gt0 += c
                nc.vector.memset(g_bufs[1][:], 0.0)
                nc.vector.memset(g_bufs[2][:], 0.0)
                if gt0 < GT_MAX:
                    nc.vector.memset(g_bufs[0][:, gt0:, :], 0.0)

                # batched one-hot builds (one DVE op each):
                # b_all[p, w, g] = (brel[p, w] == g)
                in0 = bass.AP(brel_t[:].tensor, brel_t[:].offset,
                              [list(brel_t[:].ap[0]), [1, NW], [0, gpc]])
                in1 = bass.AP(iotag_t[:].tensor, iotag_t[:].offset,
                              [list(iotag_t[:].ap[0]), [0, NW], [1, gpc]])
                nc.vector.tensor_tensor(out=b_all[:], in0=in0, in1=in1,
                                        op=mybir.AluOpType.is_equal)
                # bT_all[g, j] = (brel_flat[j] == g), staged in 16-window
                # chunks to fit SBUF next to the G buffers
                CHW = 16
                with tc.tile_pool(name="brp", bufs=1) as brpool:
                    for h, j0 in enumerate(range(0, NW, CHW)):
                        j1 = min(j0 + CHW, NW)
                        cols = (j1 - j0) * P
                        brelr_t = brpool.tile([gpc, CHW * P], F16,
                                              tag="brl", name=f"brl{h}")
                        nc.sync.dma_start(
                            out=brelr_t[:, :cols],
                            in_=brelr_d[:, j0 * P:j1 * P])
                        nc.vector.tensor_tensor(
                            out=bT_all[:, j0:j1, :],
                            in0=brelr_t[:, :cols],
                            in1=iotagc_t[:, 0:1].to_broadcast([gpc, cols]),
                            op=mybir.AluOpType.is_equal)

                psum_s = pspool.tile([gpc, D], F32, space="PSUM")

                for g in range(NGRP):
                    w0, w1 = gb[g], gb[g + 1]
                    gbase = rt0[g * n_cls]  # first tile of this group
                    g_t = g_bufs[g % 3]
                    for kk in range(n_cls):
                        capt = capt_gk[g * n_cls + kk]
                        if capt == 0:
                            continue
                        rbase = rt0[g * n_cls + kk]
                        nc.gpsimd.load(
                            gcnt,
                            cnt_t[0:1, g * n_cls + kk:g * n_cls + kk + 1])
                        base = CLASS_SIZE * kk
                        nc.gpsimd.dma_gather(
                            out_ap=g_t[:, rbase - gbase:rbase - gbase + capt,
                                       :],
                            in_ap=hfull_d[base:base + cls_size[kk], :],
                            idxs_ap=idxg[g][:, (rbase - gbase) * 8:
                                            (rbase - gbase + capt) * 8],
                            num_idxs=capt * P,
                            num_idxs_reg=gcnt,
                            elem_size=D,
                            single_packet=False,
                            queue_num=kk % 4,
                        )

                    for wq in range(w0, w1, QW):
                        q = min(QW, w1 - wq)
                        psum_w = pwpool.tile([P, QW, D], F32, space="PSUM",
                                             tag="pw")
                        for j in range(q):
                            w = wq + j
                            runs = []
                            for kk in range(n_cls):
                                lo, hi = (lo_wk[w * n_cls + kk],
                                          hi_wk[w * n_cls + kk])
                                if hi > lo:
                                    runs.append((lo, hi))
                            for ri, (lo, hi) in enumerate(runs):
                                nt = hi - lo
                                r_t = rpool.tile([P, NTMAX, P], F16, tag="R")
                                in0 = drel_t[:, lo:hi].to_broadcast(
                                    [P, nt, P])
                                sl = iota_t[:, 128 * (w % WMOD):
                                            128 * (w % WMOD) + P]
                                in1 = bass.AP(
                                    sl.tensor, sl.offset,
                                    [list(sl.ap[0]), [0, nt], list(sl.ap[1])])
                                nc.vector.tensor_tensor(
                                    out=r_t[:, :nt, :], in0=in0, in1=in1,
                                    op=mybir.AluOpType.is_equal)
                                last_run = ri == len(runs) - 1
                                for t in range(nt):
                                    nc.tensor.matmul(
                                        psum_w[:, j, :],
                                        r_t[:, t, :],
                                        g_t[:, lo - gbase + t, :],
                                        start=(ri == 0 and t == 0),
                                        stop=(last_run and t == nt - 1),
                                        skip_group_check=True)
                        nc.vector.tensor_tensor(
                            out=out_sb[:, wq:wq + q, :],
                            in0=psum_w[:, :q, :],
                            in1=out_sb[:, wq:wq + q, :],
                            op=mybir.AluOpType.add)
                        psum_oT = ptpool.tile([P, QW, P], F16, space="PSUM",
                                              tag="poT")
                        for j in range(q):
                            w = wq + j
                            nc.tensor.matmul(
                                psum_s[:], b_all[:, w, :], out_sb[:, w, :],
                                start=(w == 0), stop=(w == NW - 1),
                                skip_group_check=True)
                            nc.tensor.transpose(psum_oT[:, j, :],
                                                out_sb[:, w, :], ident_t[:])
                        oT_q = rpool.tile([P, QW, P], F16, tag="oT")
                        nc.scalar.copy(oT_q[:, :q, :], psum_oT[:, :q, :])
                        # y1 = out @ W computed here (PE has slack); out_sb
                        # rows are dead after psum_s + transpose, so y1
                        # overwrites them in f16. The tail then only needs
                        # bT @ vW + identity-add of y1.
                        psum_y1 = ptpool.tile([P, QW, D], F32, space="PSUM",
                                              tag="py1")
                        for j in range(q):
                            nc.tensor.matmul(psum_y1[:, j, :],
                                             oT_q[:, j, :], w16_t[:],
                                             start=True, stop=True,
                                             skip_group_check=True)
                        nc.scalar.copy(out_sb[:, wq:wq + q, :],
                                       psum_y1[:, :q, :])

                vmean16 = cpool.tile([gpc, D], F16)
                nc.scalar.mul(vmean16[:], psum_s[:], invc_t[:, 0:1])

            with tc.tile_pool(name="p3", bufs=4) as p3, \
                 tc.tile_pool(name="pp1", bufs=1, space="PSUM") as pp1, \
                 tc.tile_pool(name="pp3", bufs=4, space="PSUM") as pp3:
                psum_vmT = pp1.tile([P, gpc], F16, space="PSUM", tag="pvmT")
                nc.tensor.transpose(psum_vmT[:], vmean16[:],
                                    ident_t[0:gpc, 0:gpc])
                vmT = p3.tile([P, gpc], F16, tag="vmT")
                nc.scalar.copy(vmT[:], psum_vmT[:])
                psum_vw = pp1.tile([gpc, D], F32, space="PSUM", tag="pvw")
                nc.tensor.matmul(psum_vw[:], vmT[:], w16_t[:],
                                 start=True, stop=True)
                nc.scalar.copy(vw_sb[:], psum_vw[:])

                for qi, w0 in enumerate(range(0, NW, QW)):
                    q = min(QW, NW - w0)
                    psum_yq = pp3.tile([P, QW, D], F32, space="PSUM",
                                       tag="py")
                    for j in range(q):
                        w = w0 + j
                        nc.tensor.matmul(psum_yq[:, j, :], bT_all[:, w, :],
                                         vw_sb[:], start=True, stop=True,
                                         skip_group_check=True)
                    # += y1 (stored in out_sb) on DVE (idle at the tail),
                    # relu on Scalar, DMA alternating both HWDGE queues
                    ys_t = p3.tile([P, QW, D], F32, tag="YS")
                    nc.vector.tensor_tensor(
                        out=ys_t[:, :q, :], in0=psum_yq[:, :q, :],
                        in1=out_sb[:, w0:w0 + q, :],
                        op=mybir.AluOpType.add)
                    yq_t = p3.tile([P, QW, D], F32, tag="Y")
                    nc.scalar.activation(yq_t[:, :q, :], ys_t[:, :q, :],
                                         mybir.ActivationFunctionType.Relu)
                    out_ap = bass.AP(
                        y_d[:].tensor, w0 * P * D,
                        [[D, P], [P * D, q], [1, D]])
                    eng = nc.sync if qi % 2 == 0 else nc.scalar
                    eng.dma_start(out=out_ap, in_=yq_t[:, :q, :])
    _finish_compile(nc)
    return nc


def _finish_compile(nc):
    nc.compile()
    # compile()'s tail passes (library-load insertion for the custom DMA
    # instructions) can reintroduce >1 sync wait per instruction, which the
    # TRN2 ISA rejects. Re-split and re-codegen.
    import bass_rust
    bass_rust.generate_event_semaphores(nc)
    nc.codegen_inst_isa_subclasses()


_BUILD_CACHE = {}


def _build_cached(params):
    key = tuple(sorted((k, v) for k, v in params.items()))
    if key not in _BUILD_CACHE:
        _BUILD_CACHE[key] = _build(params)
    return _BUILD_CACHE[key]


def _run(H, edge_index, batch, W, n_graphs, trace=False):
    H = np.asarray(H)
    params, in_maps, n_c, core_start = _prep(H, edge_index, batch, n_graphs)
    consts = _consts(params, np.asarray(W))
    hfull16 = np.ascontiguousarray(H.astype(np.float16))
    for m in in_maps:
        m["hfull16"] = hfull16
        m.update(consts)
    nc = _build_cached(params)
    res = run_bass_kernel_spmd(nc, in_maps, list(range(N_CORES)), trace=trace)
    N = H.shape[0]
    y = np.empty((N, D), dtype=np.float32)
    for c in range(N_CORES):
        y[core_start[c]:core_start[c] + n_c[c]] = \
            res.results[c]["y"][:n_c[c]]
    return y, res


def kernel(H, edge_index, batch, W):
    y, _ = _run(H, edge_index, batch, W, n_graphs=256,
                trace=bool(os.environ.get("GCN_TRACE")))
    return y


# revision 54
# speedup vs baseline: 1.0243x; 1.0009x over previous
"""GCN layer with virtual node on 8 Trainium2 NeuronCores (Bass/Tile).

Reference computation (fp32):
    agg = segment_sum(H[src], dst, N)        # message passing
    out = H + agg
    vmean = segment_mean(out, batch, G)      # virtual node
    out = out + vmean[batch]
    y = relu(out @ W)

Distribution strategy (self-contained, hardcoded):
  - batch is sorted, G=256 graphs, 8 cores -> graph-aligned node sharding
    (cuts chosen to minimize the max per-core edge count subject to a node
    cap). Per-graph means never cross cores: no collectives needed.
  - Edges are partitioned by owning core of dst, sorted by (group, src
    class, window, src) and packed densely into per-(group, class) regions
    (class = src // 25000, int16 gather reach; 4 exactly-equal classes).
    One dma_gather per (group, class) on SWDGE queue = class: Q7
    descriptor generation (~8 ns/desc per queue) is the hard wall, and
    queue q is generated by Q7 pair (2q, 2q+1), so the 4 classes generate
    on all 8 Q7 cores in parallel (the single-queue baseline used 2).
    Tiny warm-up gathers absorb the one-time Q7 IRAM library load; G
    buffers are triple-buffered so the gather stream never stalls on
    consumers (gathers run back-to-back, ~0 idle).
  - segment_sum via PE one-hot matmul: slot s carries tag (dst&127) +
    128*(w%16); R[s, m] = (tag[s] == iota), built on DVE in one batched op
    per (window, class) run over the window's static tile range (min/max
    over cores; the mod-16 tag zeroes foreign-window slots in shared or
    drifted tiles).
  - virtual node: b_all[dst, w, g] = (batch_rel == g) built in ONE DVE op;
    psum_s[g, f] += b_all_w^T @ out_w over all windows; vmean = psum_s *
    (1/cnt) on Scalar; vW = vmean @ W once. bT[g, dst] built directly on
    DVE from a partition-replicated batch vector (no PE transposes).
  - y1 = out @ W is computed inside the window loop (PE slack) via a
    PE transpose of out_w; y1 overwrites the dead out_sb rows in f16.
    Final stage per window: psum = bT_w @ vW, DVE adds y1, Scalar relus,
    output DMA alternates both HWDGE queues. PSUM->SBUF copies are
    quad-batched (4 windows per PSUM bank) and run on Scalar.
"""
import os
import numpy as np

from concourse import bass, bacc, mybir
import concourse.tile as tile
from concourse.bass_utils import run_bass_kernel_spmd

P = 128
N_CORES = 8
D = 128
F32 = mybir.dt.float32
F16 = mybir.dt.float16
I32 = mybir.dt.int32
I16 = mybir.dt.int16
GROUP_W = 8           # windows per gather group
CLASS_SIZE = 25000    # int16 index reach; 4 exactly-equal classes -> 4 queues
WMOD = 16             # window disambiguation modulus in drel encoding
QW = 4                # windows per PSUM quad


def _ceil(a, b):
    return -(-a // b)


# ---------------------------------------------------------------------------
# host-side prep: pure index arithmetic / sharding metadata
# ---------------------------------------------------------------------------

def _prep(H, edge_index, batch, n_graphs):
    N = H.shape[0]
    src = np.asarray(edge_index[0], dtype=np.int64)
    dst = np.asarray(edge_index[1], dtype=np.int64)
    batch = np.asarray(batch, dtype=np.int64)
    n_cls = _ceil(N, CLASS_SIZE)

    gstart = np.searchsorted(batch, np.arange(n_graphs + 1))
    counts = np.diff(gstart)

    # graph-aligned cuts minimizing the max per-core edge count (the Q7
    # descriptor-generation wall) subject to a per-core node cap that keeps
    # NW at its uniform-cut value.
    epg = np.bincount(batch[dst], minlength=n_graphs).astype(np.int64)
    npg = counts.astype(np.int64)
    ncap = _ceil(int(np.diff(gstart[::n_graphs // N_CORES]).max()), P) * P

    def cuts_for(bound):
        cut = [0]
        ce = cn = 0
        for gi in range(n_graphs):
            if ce + epg[gi] > bound or cn + npg[gi] > ncap:
                if len(cut) == N_CORES:
                    return None
                cut.append(gi)
                ce = cn = 0
            ce += epg[gi]
            cn += npg[gi]
        return cut + [n_graphs] if len(cut) == N_CORES else None

    lo_b, hi_b = int(epg.sum() // N_CORES), int(epg.sum())
    while lo_b < hi_b:
        mid = (lo_b + hi_b) // 2
        if cuts_for(mid) is None:
            lo_b = mid + 1
        else:
            hi_b = mid
    best = cuts_for(lo_b)
    if best is None:  # node cap infeasible for greedy: uniform fallback
        best = list(range(0, n_graphs, n_graphs // N_CORES))[:N_CORES] \
            + [n_graphs]
    gcut = np.asarray(best, dtype=np.int64)
    gpc = int(np.diff(gcut).max())  # max graphs per core (tile dim)
    core_start = gstart[gcut]

    gcore = (np.searchsorted(gcut, batch, side="right") - 1).astype(np.int64)
    ecore = gcore[dst]

    n_c = np.diff(core_start)
    NW = int(_ceil(n_c.max(), P))
    # group sizes: small head groups (fast pipeline ramp), GROUP_W middle,
    # tapered tail (short drain: the last group's scatter work trails the
    # final gather)
    sizes = []
    rem = NW
    for s in (4,):
        if rem > s + 12:
            sizes.append(s)
            rem -= s
    tail = [s for s in (4, 2) if s < rem]
    rem -= sum(tail)
    sizes += [GROUP_W] * (rem // GROUP_W)
    if rem % GROUP_W:
        sizes.append(rem % GROUP_W)
    sizes += tail
    gb = [0]
    for s in sizes:
        gb.append(gb[-1] + s)
    assert gb[-1] == NW
    NGRP = len(gb) - 1
    grpmap = np.searchsorted(np.asarray(gb[1:]), np.arange(NW), side='right')

    percore = []
    cnt_gk = np.zeros((N_CORES, NGRP, n_cls), dtype=np.int64)
    for c in range(N_CORES):
        m = ecore == c
        s_c, d_c = src[m], dst[m]
        dstl = d_c - core_start[c]
        w = dstl >> 7
        k = s_c // CLASS_SIZE
        g = grpmap[w]
        order = np.lexsort((s_c, w, k, g))
        s_c, dstl, w, k, g = (s_c[order], dstl[order], w[order], k[order],
                              g[order])
        gk = g * n_cls + k
        cnt_gk[c] = np.bincount(gk, minlength=NGRP * n_cls).reshape(
            NGRP, n_cls)
        percore.append((s_c, dstl, w, k, g))

    cap_gk = (_ceil(cnt_gk.max(axis=0), P) * P).astype(np.int64)  # [NGRP,ncls]
    capt_gk = cap_gk // P
    rt0 = np.concatenate([[0], np.cumsum(capt_gk.ravel())]).reshape(
        -1)  # region tile starts, flat (g, k) order
    TT = int(rt0[-1])

    lo_wk = np.full((NW, n_cls), 1 << 30, dtype=np.int64)
    hi_wk = np.zeros((NW, n_cls), dtype=np.int64)
    in_maps = []
    for c in range(N_CORES):
        s_c, dstl, w, k, g_arr = percore[c]
        idx_flat = np.full(TT * P, -1, dtype=np.int64)
        drel = np.full((P, TT), -1.0, dtype=np.float16)
        tag = ((dstl & 127) + 128 * (w % WMOD)).astype(np.float16)
        for g in range(NGRP):
            w0, w1 = gb[g], gb[g + 1]
            for kk in range(n_cls):
                base_slot = int(rt0[g * n_cls + kk]) * P
                sel = (g_arr == g) & (k == kk)
                n = int(sel.sum())
                gs = base_slot + np.arange(n)
                idx_flat[gs] = s_c[sel] - CLASS_SIZE * kk
                drel[gs % P, gs // P] = tag[sel]
                if n == 0:
                    idx_flat[base_slot] = 0  # cnt>=1 pad (drel=-1)
                # per-window runs -> static tile ranges
                cnt_w = np.bincount(w[sel] - w0, minlength=w1 - w0)
                off = base_slot + np.concatenate(
                    [[0], np.cumsum(cnt_w)]).astype(np.int64)
                for wi in range(w0, w1):
                    a, b = off[wi - w0], off[wi - w0 + 1]
                    if b > a:
                        lo_wk[wi, kk] = min(lo_wk[wi, kk], a // P)
                        hi_wk[wi, kk] = max(hi_wk[wi, kk], _ceil(b, P))

        # wrap int16 indices into [16, cap/16] blocks per (g, k) region,
        # replicated to 128 partitions
        wrapped = np.zeros((P, TT * P // 16), dtype=np.int16)
        for g in range(NGRP):
            for kk in range(n_cls):
                ccap = int(cap_gk[g, kk])
                if ccap == 0:
                    continue
                sbase = int(rt0[g * n_cls + kk]) * P
                block = idx_flat[sbase:sbase + ccap]
                wb = block.reshape(ccap // 16, 16).T.astype(np.int16)
                col0 = sbase // 16
                wrapped[:16, col0:col0 + ccap // 16] = wb
        wrapped[16:] = np.tile(wrapped[:16], (7, 1))

        nodes = int(n_c[c])
        hcore = np.zeros((NW * P, D), dtype=np.float32)
        hcore[:nodes] = H[core_start[c]:core_start[c] + nodes]
        hcp = np.ascontiguousarray(
            hcore.reshape(NW, P, D).transpose(1, 0, 2)).astype(np.float16)
        brel = np.full((NW * P,), -1.0, dtype=np.float16)
        brel[:nodes] = (batch[core_start[c]:core_start[c] + nodes]
                        - gcut[c]).astype(np.float16)
        brel2 = brel.reshape(NW, P).T.copy()
        brelr = np.ascontiguousarray(
            np.broadcast_to(brel.reshape(1, NW * P), (gpc, NW * P)))
        invc = np.ones((gpc, 1), dtype=np.float32)
        ngr = int(gcut[c + 1] - gcut[c])
        invc[:ngr, 0] = 1.0 / np.maximum(counts[gcut[c]:gcut[c + 1]], 1)

        in_maps.append({
            "idx16": np.ascontiguousarray(wrapped),
            "cnt": np.ascontiguousarray(
                np.maximum(cnt_gk[c], 1).reshape(
                    1, NGRP * n_cls).astype(np.int32)),
            "drel": np.ascontiguousarray(drel),
            "hcp": hcp,
            "brel": np.ascontiguousarray(brel2),
            "brelr": brelr,
            "invc": invc,
        })

    for wi in range(NW):
        for kk in range(n_cls):
            if lo_wk[wi, kk] >= (1 << 30):
                lo_wk[wi, kk] = hi_wk[wi, kk] = 0
    # every window must have at least one nonempty run (else PSUM quad
    # slices would stay uninitialized)
    for wi in range(NW):
        assert any(hi_wk[wi, kk] > lo_wk[wi, kk] for kk in range(n_cls)), wi
    # mod-WMOD safety: same-residue windows must not have overlapping spans
    for kk in range(n_cls):
        for wi in range(NW):
            for wj in range(wi + 1, NW):
                if wi % WMOD == wj % WMOD:
                    if not (hi_wk[wj, kk] <= lo_wk[wi, kk]
                            or lo_wk[wj, kk] >= hi_wk[wi, kk]):
                        raise RuntimeError(
                            f"mod-{WMOD} window collision {wi},{wj},{kk}")

    params = dict(
        N=N, NW=NW, NGRP=NGRP, gpc=gpc, TT=TT, n_cls=n_cls,
        gb=tuple(int(x) for x in gb),
        capt_gk=tuple(int(x) for x in capt_gk.ravel()),
        rt0=tuple(int(x) for x in rt0),
        lo_wk=tuple(int(x) for x in lo_wk.ravel()),
        hi_wk=tuple(int(x) for x in hi_wk.ravel()),
        cls_size=tuple(min(CLASS_SIZE, N - CLASS_SIZE * kk)
                       for kk in range(n_cls)),
    )
    return params, in_maps, n_c, core_start


def _consts(params, W):
    iota1024 = np.broadcast_to(
        np.arange(WMOD * P, dtype=np.float16), (P, WMOD * P)).copy()
    warmix = np.zeros((P, 8), dtype=np.int16)
    iotag = np.broadcast_to(
        np.arange(params["gpc"], dtype=np.float16), (P, params["gpc"])).copy()
    iotagc = np.arange(params["gpc"], dtype=np.float16).reshape(-1, 1).copy()
    ident = np.eye(P, dtype=np.float16)
    return {"iota1024": iota1024, "warmix": warmix,
            "iotag": iotag, "iotagc": iotagc,
            "ident": ident, "w16": np.ascontiguousarray(W, dtype=np.float16)}


# ---------------------------------------------------------------------------
# device kernel builder (SPMD: one program, per-core data)
# ---------------------------------------------------------------------------

def _build(params):
    NW, NGRP, TT = params["NW"], params["NGRP"], params["TT"]
    gpc, n_cls = params["gpc"], params["n_cls"]
    capt_gk = params["capt_gk"]
    gb = params["gb"]
    rt0 = params["rt0"]
    lo_wk, hi_wk = params["lo_wk"], params["hi_wk"]
    cls_size = params["cls_size"]
    N = params["N"]
    NTMAX = int(max(max(hi_wk[i] - lo_wk[i] for i in range(NW * n_cls)), 1))
    GRP_TILES = [sum(capt_gk[g * n_cls + kk] for kk in range(n_cls))
                 for g in range(NGRP)]
    GT_MAX = max(GRP_TILES)

    nc = bacc.Bacc("TRN2", target_bir_lowering=False, debug=False,
                   num_devices=N_CORES, num_swdge_queues=4)
    hfull_d = nc.dram_tensor("hfull16", [N, D], F16, kind="ExternalInput")
    idx_d = nc.dram_tensor("idx16", [P, TT * P // 16], I16,
                           kind="ExternalInput")
    cnt_d = nc.dram_tensor("cnt", [1, NGRP * n_cls], I32,
                           kind="ExternalInput")
    warmix_d = nc.dram_tensor("warmix", [P, 8], I16, kind="ExternalInput")
    drel_d = nc.dram_tensor("drel", [P, TT], F16, kind="ExternalInput")
    hcp_d = nc.dram_tensor("hcp", [P, NW, D], F16, kind="ExternalInput")
    brel_d = nc.dram_tensor("brel", [P, NW], F16, kind="ExternalInput")
    brelr_d = nc.dram_tensor("brelr", [gpc, NW * P], F16,
                             kind="ExternalInput")
    invc_d = nc.dram_tensor("invc", [gpc, 1], F32, kind="ExternalInput")
    iota_d = nc.dram_tensor("iota1024", [P, WMOD * P], F16,
                            kind="ExternalInput")
    iotag_d = nc.dram_tensor("iotag", [P, gpc], F16, kind="ExternalInput")
    iotagc_d = nc.dram_tensor("iotagc", [gpc, 1], F16, kind="ExternalInput")
    ident_d = nc.dram_tensor("ident", [P, P], F16, kind="ExternalInput")
    w16_d = nc.dram_tensor("w16", [P, D], F16, kind="ExternalInput")
    y_d = nc.dram_tensor("y", [NW * P, D], F32, kind="ExternalOutput")

    with tile.TileContext(nc) as tc:
        with tc.tile_pool(name="const", bufs=1) as cpool, \
             nc.gpsimd.register("gcnt") as gcnt:
            cnt_t = cpool.tile([1, NGRP * n_cls], I32)
            nc.sync.dma_start(out=cnt_t[:], in_=cnt_d[:])
            warmix_t = cpool.tile([P, 8], I16)
            nc.sync.dma_start(out=warmix_t[:], in_=warmix_d[:])
            idxg = []
            gspan = []
            for g in range(NGRP):
                c0 = rt0[g * n_cls] * 8
                c1 = rt0[(g + 1) * n_cls] * 8
                gspan.append((c0, c1))
                idxg.append(cpool.tile([P, c1 - c0], I16, name=f"idxg{g}"))
            # group-0 indices first so the first gather starts early; the
            # remaining groups' index loads hide under gen.
            nc.sync.dma_start(out=idxg[0][:], in_=idx_d[:, gspan[0][0]:
                                                        gspan[0][1]])
            iota_t = cpool.tile([P, WMOD * P], F16)
            nc.sync.dma_start(out=iota_t[:], in_=iota_d[:])
            iotag_t = cpool.tile([P, gpc], F16)
            nc.sync.dma_start(out=iotag_t[:], in_=iotag_d[:])
            iotagc_t = cpool.tile([gpc, 1], F16)
            nc.sync.dma_start(out=iotagc_t[:], in_=iotagc_d[:])
            ident_t = cpool.tile([P, P], F16)
            nc.sync.dma_start(out=ident_t[:], in_=ident_d[:])
            w16_t = cpool.tile([P, D], F16)
            nc.sync.dma_start(out=w16_t[:], in_=w16_d[:])
            invc_t = cpool.tile([gpc, 1], F32)
            nc.sync.dma_start(out=invc_t[:], in_=invc_d[:])
            brel_t = cpool.tile([P, NW], F16)
            nc.sync.dma_start(out=brel_t[:], in_=brel_d[:])
            drel_t = cpool.tile([P, TT], F16)
            nc.sync.dma_start(out=drel_t[:], in_=drel_d[:])
            out_sb = cpool.tile([P, NW, D], F16)
            nc.sync.dma_start(out=out_sb[:], in_=hcp_d[:])
            for g in range(1, NGRP):
                nc.sync.dma_start(out=idxg[g][:],
                                  in_=idx_d[:, gspan[g][0]:gspan[g][1]])
            b_all = cpool.tile([P, NW, gpc], F16)
            bT_all = cpool.tile([gpc, NW, P], F16)
            vw_sb = cpool.tile([gpc, D], F16)

            with tc.tile_pool(name="gpool", bufs=3) as gpool, \
                 tc.tile_pool(name="rpool", bufs=4) as rpool, \
                 tc.tile_pool(name="pw", bufs=3, space="PSUM") as pwpool, \
                 tc.tile_pool(name="pt", bufs=2, space="PSUM") as ptpool, \
                 tc.tile_pool(name="ps", bufs=1, space="PSUM") as pspool:

                # G buffers memset FIRST on DVE (no input deps) so the
                # first two groups' gathers are not stalled behind the
                # one-hot builds or group-0's R work. Every byte of both
                # buffers must be finite before first use: static (min/max
                # over cores) tile ranges may read slots this core never
                # gathers.
                g_bufs = [gpool.tile([P, GT_MAX, D], F16, tag="G",
                                     name=f"gbuf{i}")
                          for i in range(3)]
                # warm each queue's Q7 pair (one-time ~6us IRAM library
                # load) with a tiny 128-idx gather into scratch before the
                # real stream begins
                for kk in range(min(n_cls, 4)):
                    nc.gpsimd.dma_gather(
                        out_ap=g_bufs[2][:, kk:kk + 1, :],
                        in_ap=hfull_d[0:cls_size[0], :],
                        idxs_ap=warmix_t[:],
                        num_idxs=P,
                        num_idxs_reg=P,
                        elem_size=D,
                        single_packet=False,
                        queue_num=kk % 4,
                    )
                # first-group memset split per class region so gather (0,k)
                # only waits for its own piece
                gt0 = 0
                for kk in range(n_cls):
                    c = capt_gk[kk]
                    if c:
                        nc.vector.memset(g_bufs[0][:, gt0:gt0 + c, :], 0.0)
                    gt0 += c
                nc.vector.memset(g_bufs[1][:], 0.0)
                nc.vector.memset(g_bufs[2][:], 0.0)
                if gt0 < GT_MAX:
                    nc.vector.memset(g_bufs[0][:, gt0:, :], 0.0)

                # batched one-hot builds (one DVE op each):
                # b_all[p, w, g] = (brel[p, w] == g)
                in0 = bass.AP(brel_t[:].tensor, brel_t[:].offset,
                              [list(brel_t[:].ap[0]), [1, NW], [0, gpc]])
                in1 = bass.AP(iotag_t[:].tensor, iotag_t[:].offset,
                              [list(iotag_t[:].ap[0]), [0, NW], [1, gpc]])
                nc.vector.tensor_tensor(out=b_all[:], in0=in0, in1=in1,
                                        op=mybir.AluOpType.is_equal)
                # bT_all[g, j] = (brel_flat[j] == g), staged in 16-window
                # chunks to fit SBUF next to the G buffers
                CHW = 16
                with tc.tile_pool(name="brp", bufs=1) as brpool:
                    for h, j0 in enumerate(range(0, NW, CHW)):
                        j1 = min(j0 + CHW, NW)
                        cols = (j1 - j0) * P
                        brelr_t = brpool.tile([gpc, CHW * P], F16,
                                              tag="brl", name=f"brl{h}")
                        nc.sync.dma_start(
                            out=brelr_t[:, :cols],
                            in_=brelr_d[:, j0 * P:j1 * P])
                        nc.vector.tensor_tensor(
                            out=bT_all[:, j0:j1, :],
                            in0=brelr_t[:, :cols],
                            in1=iotagc_t[:, 0:1].to_broadcast([gpc, cols]),
                            op=mybir.AluOpType.is_equal)

                psum_s = pspool.tile([gpc, D], F32, space="PSUM")

                for g in range(NGRP):
                    w0, w1 = gb[g], gb[g + 1]
                    gbase = rt0[g * n_cls]  # first tile of this group
                    g_t = g_bufs[g % 3]
                    for kk in range(n_cls):
                        capt = capt_gk[g * n_cls + kk]
                        if capt == 0:
                            continue
                        rbase = rt0[g * n_cls + kk]
                        nc.gpsimd.load(
                            gcnt,
                            cnt_t[0:1, g * n_cls + kk:g * n_cls + kk + 1])
                        base = CLASS_SIZE * kk
                        nc.gpsimd.dma_gather(
                            out_ap=g_t[:, rbase - gbase:rbase - gbase + capt,
                                       :],
                            in_ap=hfull_d[base:base + cls_size[kk], :],
                            idxs_ap=idxg[g][:, (rbase - gbase) * 8:
                                            (rbase - gbase + capt) * 8],
                            num_idxs=capt * P,
                            num_idxs_reg=gcnt,
                            elem_size=D,
                            single_packet=False,
                            queue_num=kk % 4,
                        )

                    for wq in range(w0, w1, QW):
                        q = min(QW, w1 - wq)
                        psum_w = pwpool.tile([P, QW, D], F32, space="PSUM",
                                             tag="pw")
                        for j in range(q):
                            w = wq + j
                            runs = []
                            for kk in range(n_cls):
                                lo, hi = (lo_wk[w * n_cls + kk],
                                          hi_wk[w * n_cls + kk])
                                if hi > lo:
                                    runs.append((lo, hi))
                            for ri, (lo, hi) in enumerate(runs):
                                nt = hi - lo
                                r_t = rpool.tile([P, NTMAX, P], F16, tag="R")
                                in0 = drel_t[:, lo:hi].to_broadcast(
                                    [P, nt, P])
                                sl = iota_t[:, 128 * (w % WMOD):
                                            128 * (w % WMOD) + P]
                                in1 = bass.AP(
                                    sl.tensor, sl.offset,
                                    [list(sl.ap[0]), [0, nt], list(sl.ap[1])])
                                nc.vector.tensor_tensor(
                                    out=r_t[:, :nt, :], in0=in0, in1=in1,
                                    op=mybir.AluOpType.is_equal)
                                last_run = ri == len(runs) - 1
                                for t in range(nt):
                                    nc.tensor.matmul(
                                        psum_w[:, j, :],
                                        r_t[:, t, :],
                                        g_t[:, lo - gbase + t, :],
                                        start=(ri == 0 and t == 0),
                                        stop=(last_run and t == nt - 1),
                                        skip_group_check=True)
                        nc.vector.tensor_tensor(
                            out=out_sb[:, wq:wq + q, :],
                            in0=psum_w[:, :q, :],
                            in1=out_sb[:, wq:wq + q, :],
                            op=mybir.AluOpType.add)
                        psum_oT = ptpool.tile([P, QW, P], F16, space="PSUM",
                                              tag="poT")
                        for j in range(q):
                            w = wq + j
                            nc.tensor.matmul(
                                psum_s[:], b_all[:, w, :], out_sb[:, w, :],
                                start=(w == 0), stop=(w == NW - 1),
                                skip_group_check=True)
                            nc.tensor.transpose(psum_oT[:, j, :],
                                                out_sb[:, w, :], ident_t[:])
                        oT_q = rpool.tile([P, QW, P], F16, tag="oT")
                        nc.scalar.copy(oT_q[:, :q, :], psum_oT[:, :q, :])
                        # y1 = out @ W computed here (PE has slack); out_sb
                        # rows are dead after psum_s + transpose, so y1
                        # overwrites them in f16. The tail then only needs
                        # bT @ vW + identity-add of y1.
                        psum_y1 = ptpool.tile([P, QW, D], F32, space="PSUM",
                                              tag="py1")
                        for j in range(q):
                            nc.tensor.matmul(psum_y1[:, j, :],
                                             oT_q[:, j, :], w16_t[:],
                                             start=True, stop=True,
                                             skip_group_check=True)
                        nc.scalar.copy(out_sb[:, wq:wq + q, :],
                                       psum_y1[:, :q, :])

                vmean16 = cpool.tile([gpc, D], F16)
                nc.scalar.mul(vmean16[:], psum_s[:], invc_t[:, 0:1])

            with tc.tile_pool(name="p3", bufs=4) as p3, \
                 tc.tile_pool(name="pp1", bufs=1, space="PSUM") as pp1, \
                 tc.tile_pool(name="pp3", bufs=4, space="PSUM") as pp3:
                psum_vmT = pp1.tile([P, gpc], F16, space="PSUM", tag="pvmT")
                nc.tensor.transpose(psum_vmT[:], vmean16[:],
                                    ident_t[0:gpc, 0:gpc])
                vmT = p3.tile([P, gpc], F16, tag="vmT")
                nc.scalar.copy(vmT[:], psum_vmT[:])
                psum_vw = pp1.tile([gpc, D], F32, space="PSUM", tag="pvw")
                nc.tensor.matmul(psum_vw[:], vmT[:], w16_t[:],
                                 start=True, stop=True)
                nc.scalar.copy(vw_sb[:], psum_vw[:])

                for qi, w0 in enumerate(range(0, NW, QW)):
                    q = min(QW, NW - w0)
                    psum_yq = pp3.tile([P, QW, D], F32, space="PSUM",
                                       tag="py")
                    for j in range(q):
                        w = w0 + j
                        nc.tensor.matmul(psum_yq[:, j, :], bT_all[:, w, :],
                                         vw_sb[:], start=True, stop=True,
                                         skip_group_check=True)
                    # += y1 (stored in out_sb) on DVE (idle at the tail),
                    # relu on Scalar, DMA alternating both HWDGE queues
                    ys_t = p3.tile([P, QW, D], F32, tag="YS")
                    nc.vector.tensor_tensor(
                        out=ys_t[:, :q, :], in0=psum_yq[:, :q, :],
                        in1=out_sb[:, w0:w0 + q, :],
                        op=mybir.AluOpType.add)
                    yq_t = p3.tile([P, QW, D], F32, tag="Y")
                    nc.scalar.activation(yq_t[:, :q, :], ys_t[:, :q, :],
                                         mybir.ActivationFunctionType.Relu)
                    out_ap = bass.AP(
                        y_d[:].tensor, w0 * P * D,
                        [[D, P], [P * D, q], [1, D]])
                    eng = nc.sync if qi % 2 == 0 else nc.scalar
                    eng.dma_start(out=out_ap, in_=yq_t[:, :q, :])
    _finish_compile(nc)
    return nc


def _finish_compile(nc):
    nc.compile()
    # compile()'s tail passes (library-load insertion for the custom DMA
    # instructions) can reintroduce >1 sync wait per instruction, which the
    # TRN2 ISA rejects. Re-split and re-codegen.
    import bass_rust
    bass_rust.generate_event_semaphores(nc)
    nc.codegen_inst_isa_subclasses()


_BUILD_CACHE = {}


def _build_cached(params):
    key = tuple(sorted((k, v) for k, v in params.items()))
    if key not in _BUILD_CACHE:
        _BUILD_CACHE[key] = _build(params)
    return _BUILD_CACHE[key]


def _run(H, edge_index, batch, W, n_graphs, trace=False):
    H = np.asarray(H)
    params, in_maps, n_c, core_start = _prep(H, edge_index, batch, n_graphs)
    consts = _consts(params, np.asarray(W))
    hfull16 = np.ascontiguousarray(H.astype(np.float16))
    for m in in_maps:
        m["hfull16"] = hfull16
        m.update(consts)
    nc = _build_cached(params)
    res = run_bass_kernel_spmd(nc, in_maps, list(range(N_CORES)), trace=trace)
    N = H.shape[0]
    y = np.empty((N, D), dtype=np.float32)
    for c in range(N_CORES):
        y[core_start[c]:core_start[c] + n_c[c]] = \
            res.results[c]["y"][:n_c[c]]
    return y, res


def kernel(H, edge_index, batch, W):
    y, _ = _run(H, edge_index, batch, W, n_graphs=256,
                trace=bool(os.environ.get("GCN_TRACE")))
    return y


# revision 55
# speedup vs baseline: 1.0565x; 1.0314x over previous
"""GCN layer with virtual node on 8 Trainium2 NeuronCores (Bass/Tile).

Reference computation (fp32):
    agg = segment_sum(H[src], dst, N)        # message passing
    out = H + agg
    vmean = segment_mean(out, batch, G)      # virtual node
    out = out + vmean[batch]
    y = relu(out @ W)

Distribution strategy (self-contained, hardcoded):
  - batch is sorted, G=256 graphs, 8 cores -> graph-aligned node sharding
    (cuts chosen to minimize the max per-core edge count subject to a node
    cap). Per-graph means never cross cores: no collectives needed.
  - Edges are partitioned by owning core of dst, sorted by (group, src
    class, window, src) and packed densely into per-(group, class) regions
    (class = src // 25000, int16 gather reach; 4 exactly-equal classes).
    One dma_gather per (group, class) on SWDGE queue = class: Q7
    descriptor generation (~8 ns/desc per queue) is the hard wall, and
    queue q is generated by Q7 pair (2q, 2q+1), so the 4 classes generate
    on all 8 Q7 cores in parallel (the single-queue baseline used 2).
    Tiny warm-up gathers absorb the one-time Q7 IRAM library load; G
    buffers are triple-buffered so the gather stream never stalls on
    consumers (gathers run back-to-back, ~0 idle).
  - segment_sum via PE one-hot matmul: slot s carries tag (dst&127) +
    128*(w%16); R[s, m] = (tag[s] == iota), built on DVE in one batched op
    per (window, class) run over the window's static tile range (min/max
    over cores; the mod-16 tag zeroes foreign-window slots in shared or
    drifted tiles).
  - virtual node: b_all[dst, w, g] = (batch_rel == g) built in ONE DVE op;
    psum_s[g, f] += b_all_w^T @ out_w over all windows; vmean = psum_s *
    (1/cnt) on Scalar; vW = vmean @ W once. bT[g, dst] built directly on
    DVE from a partition-replicated batch vector (no PE transposes).
  - y1 = out @ W is computed inside the window loop (PE slack) via a
    PE transpose of out_w; y1 overwrites the dead out_sb rows in f16.
    Final stage per window: psum = bT_w @ vW, DVE adds y1, Scalar relus,
    output DMA alternates both HWDGE queues. PSUM->SBUF copies are
    quad-batched (4 windows per PSUM bank) and run on Scalar.
"""
import os
import numpy as np

from concourse import bass, bacc, mybir
import concourse.tile as tile
from concourse.bass_utils import run_bass_kernel_spmd

P = 128
N_CORES = 8
D = 128
F32 = mybir.dt.float32
F16 = mybir.dt.float16
I32 = mybir.dt.int32
I16 = mybir.dt.int16
GROUP_W = 8           # windows per gather group
CLASS_SIZE = 25000    # int16 index reach; 4 exactly-equal classes -> 4 queues
WMOD = 16             # window disambiguation modulus in drel encoding
QW = 4                # windows per PSUM quad


def _ceil(a, b):
    return -(-a // b)


# ---------------------------------------------------------------------------
# host-side prep: pure index arithmetic / sharding metadata
# ---------------------------------------------------------------------------

def _prep(H, edge_index, batch, n_graphs):
    N = H.shape[0]
    src = np.asarray(edge_index[0], dtype=np.int64)
    dst = np.asarray(edge_index[1], dtype=np.int64)
    batch = np.asarray(batch, dtype=np.int64)
    n_cls = _ceil(N, CLASS_SIZE)

    gstart = np.searchsorted(batch, np.arange(n_graphs + 1))
    counts = np.diff(gstart)

    # graph-aligned cuts minimizing the max per-core edge count (the Q7
    # descriptor-generation wall) subject to a per-core node cap that keeps
    # NW at its uniform-cut value.
    epg = np.bincount(batch[dst], minlength=n_graphs).astype(np.int64)
    npg = counts.astype(np.int64)
    ncap = _ceil(int(np.diff(gstart[::n_graphs // N_CORES]).max()), P) * P

    def cuts_for(bound):
        cut = [0]
        ce = cn = 0
        for gi in range(n_graphs):
            if ce + epg[gi] > bound or cn + npg[gi] > ncap:
                if len(cut) == N_CORES:
                    return None
                cut.append(gi)
                ce = cn = 0
            ce += epg[gi]
            cn += npg[gi]
        return cut + [n_graphs] if len(cut) == N_CORES else None

    lo_b, hi_b = int(epg.sum() // N_CORES), int(epg.sum())
    while lo_b < hi_b:
        mid = (lo_b + hi_b) // 2
        if cuts_for(mid) is None:
            lo_b = mid + 1
        else:
            hi_b = mid
    best = cuts_for(lo_b)
    if best is None:  # node cap infeasible for greedy: uniform fallback
        best = list(range(0, n_graphs, n_graphs // N_CORES))[:N_CORES] \
            + [n_graphs]
    gcut = np.asarray(best, dtype=np.int64)
    gpc = int(np.diff(gcut).max())  # max graphs per core (tile dim)
    core_start = gstart[gcut]

    gcore = (np.searchsorted(gcut, batch, side="right") - 1).astype(np.int64)
    ecore = gcore[dst]

    n_c = np.diff(core_start)
    NW = int(_ceil(n_c.max(), P))
    # group sizes: small head groups (fast pipeline ramp), GROUP_W middle,
    # tapered tail (short drain: the last group's scatter work trails the
    # final gather)
    sizes = []
    rem = NW
    for s in (4,):
        if rem > s + 12:
            sizes.append(s)
            rem -= s
    tail = [s for s in (4, 2) if s < rem]
    rem -= sum(tail)
    sizes += [GROUP_W] * (rem // GROUP_W)
    if rem % GROUP_W:
        sizes.append(rem % GROUP_W)
    sizes += tail
    gb = [0]
    for s in sizes:
        gb.append(gb[-1] + s)
    assert gb[-1] == NW
    NGRP = len(gb) - 1
    grpmap = np.searchsorted(np.asarray(gb[1:]), np.arange(NW), side='right')

    percore = []
    cnt_gk = np.zeros((N_CORES, NGRP, n_cls), dtype=np.int64)
    for c in range(N_CORES):
        m = ecore == c
        s_c, d_c = src[m], dst[m]
        dstl = d_c - core_start[c]
        w = dstl >> 7
        k = s_c // CLASS_SIZE
        g = grpmap[w]
        order = np.lexsort((s_c, w, k, g))
        s_c, dstl, w, k, g = (s_c[order], dstl[order], w[order], k[order],
                              g[order])
        gk = g * n_cls + k
        cnt_gk[c] = np.bincount(gk, minlength=NGRP * n_cls).reshape(
            NGRP, n_cls)
        percore.append((s_c, dstl, w, k, g))

    cap_gk = (_ceil(cnt_gk.max(axis=0), P) * P).astype(np.int64)  # [NGRP,ncls]
    capt_gk = cap_gk // P
    rt0 = np.concatenate([[0], np.cumsum(capt_gk.ravel())]).reshape(
        -1)  # region tile starts, flat (g, k) order
    TT = int(rt0[-1])

    lo_wk = np.full((NW, n_cls), 1 << 30, dtype=np.int64)
    hi_wk = np.zeros((NW, n_cls), dtype=np.int64)
    in_maps = []
    for c in range(N_CORES):
        s_c, dstl, w, k, g_arr = percore[c]
        idx_flat = np.full(TT * P, -1, dtype=np.int64)
        drel = np.full((P, TT), -1.0, dtype=np.float16)
        tag = ((dstl & 127) + 128 * (w % WMOD)).astype(np.float16)
        for g in range(NGRP):
            w0, w1 = gb[g], gb[g + 1]
            for kk in range(n_cls):
                base_slot = int(rt0[g * n_cls + kk]) * P
                sel = (g_arr == g) & (k == kk)
                n = int(sel.sum())
                gs = base_slot + np.arange(n)
                idx_flat[gs] = s_c[sel] - CLASS_SIZE * kk
                drel[gs % P, gs // P] = tag[sel]
                if n == 0:
                    idx_flat[base_slot] = 0  # cnt>=1 pad (drel=-1)
                # per-window runs -> static tile ranges
                cnt_w = np.bincount(w[sel] - w0, minlength=w1 - w0)
                off = base_slot + np.concatenate(
                    [[0], np.cumsum(cnt_w)]).astype(np.int64)
                for wi in range(w0, w1):
                    a, b = off[wi - w0], off[wi - w0 + 1]
                    if b > a:
                        lo_wk[wi, kk] = min(lo_wk[wi, kk], a // P)
                        hi_wk[wi, kk] = max(hi_wk[wi, kk], _ceil(b, P))

        # wrap int16 indices into [16, cap/16] blocks per (g, k) region,
        # replicated to 128 partitions
        wrapped = np.zeros((P, TT * P // 16), dtype=np.int16)
        for g in range(NGRP):
            for kk in range(n_cls):
                ccap = int(cap_gk[g, kk])
                if ccap == 0:
                    continue
                sbase = int(rt0[g * n_cls + kk]) * P
                block = idx_flat[sbase:sbase + ccap]
                wb = block.reshape(ccap // 16, 16).T.astype(np.int16)
                col0 = sbase // 16
                wrapped[:16, col0:col0 + ccap // 16] = wb
        wrapped[16:] = np.tile(wrapped[:16], (7, 1))

        nodes = int(n_c[c])
        hcore = np.zeros((NW * P, D), dtype=np.float32)
        hcore[:nodes] = H[core_start[c]:core_start[c] + nodes]
        hcp = np.ascontiguousarray(
            hcore.reshape(NW, P, D).transpose(1, 0, 2)).astype(np.float16)
        brel = np.full((NW * P,), -1.0, dtype=np.float16)
        brel[:nodes] = (batch[core_start[c]:core_start[c] + nodes]
                        - gcut[c]).astype(np.float16)
        brel2 = brel.reshape(NW, P).T.copy()
        brelr = np.ascontiguousarray(
            np.broadcast_to(brel.reshape(1, NW * P), (gpc, NW * P)))
        invc = np.ones((gpc, 1), dtype=np.float32)
        ngr = int(gcut[c + 1] - gcut[c])
        invc[:ngr, 0] = 1.0 / np.maximum(counts[gcut[c]:gcut[c + 1]], 1)

        in_maps.append({
            "idx16": np.ascontiguousarray(wrapped),
            "cnt": np.ascontiguousarray(
                np.maximum(cnt_gk[c], 1).reshape(
                    1, NGRP * n_cls).astype(np.int32)),
            "drel": np.ascontiguousarray(drel),
            "hcp": hcp,
            "brel": np.ascontiguousarray(brel2),
            "brelr": brelr,
            "invc": invc,
        })

    for wi in range(NW):
        for kk in range(n_cls):
            if lo_wk[wi, kk] >= (1 << 30):
                lo_wk[wi, kk] = hi_wk[wi, kk] = 0
    # every window must have at least one nonempty run (else PSUM quad
    # slices would stay uninitialized)
    for wi in range(NW):
        assert any(hi_wk[wi, kk] > lo_wk[wi, kk] for kk in range(n_cls)), wi
    # mod-WMOD safety: same-residue windows must not have overlapping spans
    for kk in range(n_cls):
        for wi in range(NW):
            for wj in range(wi + 1, NW):
                if wi % WMOD == wj % WMOD:
                    if not (hi_wk[wj, kk] <= lo_wk[wi, kk]
                            or lo_wk[wj, kk] >= hi_wk[wi, kk]):
                        raise RuntimeError(
                            f"mod-{WMOD} window collision {wi},{wj},{kk}")

    params = dict(
        N=N, NW=NW, NGRP=NGRP, gpc=gpc, TT=TT, n_cls=n_cls,
        gb=tuple(int(x) for x in gb),
        capt_gk=tuple(int(x) for x in capt_gk.ravel()),
        rt0=tuple(int(x) for x in rt0),
        lo_wk=tuple(int(x) for x in lo_wk.ravel()),
        hi_wk=tuple(int(x) for x in hi_wk.ravel()),
        cls_size=tuple(min(CLASS_SIZE, N - CLASS_SIZE * kk)
                       for kk in range(n_cls)),
    )
    return params, in_maps, n_c, core_start


def _consts(params, W):
    iota1024 = np.broadcast_to(
        np.arange(WMOD * P, dtype=np.float16), (P, WMOD * P)).copy()
    warmix = np.zeros((P, 8), dtype=np.int16)
    iotag = np.broadcast_to(
        np.arange(params["gpc"], dtype=np.float16), (P, params["gpc"])).copy()
    iotagc = np.arange(params["gpc"], dtype=np.float16).reshape(-1, 1).copy()
    ident = np.eye(P, dtype=np.float16)
    return {"iota1024": iota1024, "warmix": warmix,
            "iotag": iotag, "iotagc": iotagc,
            "ident": ident, "w16": np.ascontiguousarray(W, dtype=np.float16)}


# ---------------------------------------------------------------------------
# device kernel builder (SPMD: one program, per-core data)
# ---------------------------------------------------------------------------

def _build(params):
    NW, NGRP, TT = params["NW"], params["NGRP"], params["TT"]
    gpc, n_cls = params["gpc"], params["n_cls"]
    capt_gk = params["capt_gk"]
    gb = params["gb"]
    rt0 = params["rt0"]
    lo_wk, hi_wk = params["lo_wk"], params["hi_wk"]
    cls_size = params["cls_size"]
    N = params["N"]
    NTMAX = int(max(max(hi_wk[i] - lo_wk[i] for i in range(NW * n_cls)), 1))
    GRP_TILES = [sum(capt_gk[g * n_cls + kk] for kk in range(n_cls))
                 for g in range(NGRP)]
    GT_MAX = max(GRP_TILES)

    nc = bacc.Bacc("TRN2", target_bir_lowering=False, debug=False,
                   num_devices=N_CORES, num_swdge_queues=4)
    hfull_d = nc.dram_tensor("hfull16", [N, D], F16, kind="ExternalInput")
    idx_d = nc.dram_tensor("idx16", [P, TT * P // 16], I16,
                           kind="ExternalInput")
    cnt_d = nc.dram_tensor("cnt", [1, NGRP * n_cls], I32,
                           kind="ExternalInput")
    warmix_d = nc.dram_tensor("warmix", [P, 8], I16, kind="ExternalInput")
    drel_d = nc.dram_tensor("drel", [P, TT], F16, kind="ExternalInput")
    hcp_d = nc.dram_tensor("hcp", [P, NW, D], F16, kind="ExternalInput")
    brel_d = nc.dram_tensor("brel", [P, NW], F16, kind="ExternalInput")
    brelr_d = nc.dram_tensor("brelr", [gpc, NW * P], F16,
                             kind="ExternalInput")
    invc_d = nc.dram_tensor("invc", [gpc, 1], F32, kind="ExternalInput")
    iota_d = nc.dram_tensor("iota1024", [P, WMOD * P], F16,
                            kind="ExternalInput")
    iotag_d = nc.dram_tensor("iotag", [P, gpc], F16, kind="ExternalInput")
    iotagc_d = nc.dram_tensor("iotagc", [gpc, 1], F16, kind="ExternalInput")
    ident_d = nc.dram_tensor("ident", [P, P], F16, kind="ExternalInput")
    w16_d = nc.dram_tensor("w16", [P, D], F16, kind="ExternalInput")
    y_d = nc.dram_tensor("y", [NW * P, D], F32, kind="ExternalOutput")

    with tile.TileContext(nc) as tc:
        with tc.tile_pool(name="const", bufs=1) as cpool, \
             nc.gpsimd.register("gcnt") as gcnt:
            cnt_t = cpool.tile([1, NGRP * n_cls], I32)
            nc.sync.dma_start(out=cnt_t[:], in_=cnt_d[:])
            warmix_t = cpool.tile([P, 8], I16)
            nc.sync.dma_start(out=warmix_t[:], in_=warmix_d[:])
            idxg = []
            gspan = []
            for g in range(NGRP):
                c0 = rt0[g * n_cls] * 8
                c1 = rt0[(g + 1) * n_cls] * 8
                gspan.append((c0, c1))
                idxg.append(cpool.tile([P, c1 - c0], I16, name=f"idxg{g}"))
            # group-0 indices first so the first gather starts early; the
            # remaining groups' index loads hide under gen.
            nc.sync.dma_start(out=idxg[0][:], in_=idx_d[:, gspan[0][0]:
                                                        gspan[0][1]])
            iota_t = cpool.tile([P, WMOD * P], F16)
            nc.sync.dma_start(out=iota_t[:], in_=iota_d[:])
            iotag_t = cpool.tile([P, gpc], F16)
            nc.sync.dma_start(out=iotag_t[:], in_=iotag_d[:])
            iotagc_t = cpool.tile([gpc, 1], F16)
            nc.sync.dma_start(out=iotagc_t[:], in_=iotagc_d[:])
            ident_t = cpool.tile([P, P], F16)
            nc.sync.dma_start(out=ident_t[:], in_=ident_d[:])
            w16_t = cpool.tile([P, D], F16)
            nc.sync.dma_start(out=w16_t[:], in_=w16_d[:])
            invc_t = cpool.tile([gpc, 1], F32)
            nc.sync.dma_start(out=invc_t[:], in_=invc_d[:])
            brel_t = cpool.tile([P, NW], F16)
            nc.sync.dma_start(out=brel_t[:], in_=brel_d[:])
            drel_t = cpool.tile([P, TT], F16)
            nc.sync.dma_start(out=drel_t[:], in_=drel_d[:])
            out_sb = cpool.tile([P, NW, D], F16)
            nc.sync.dma_start(out=out_sb[:], in_=hcp_d[:])
            for g in range(1, NGRP):
                nc.sync.dma_start(out=idxg[g][:],
                                  in_=idx_d[:, gspan[g][0]:gspan[g][1]])
            b_all = cpool.tile([P, NW, gpc], F16)
            bT_all = cpool.tile([gpc, NW, P], F16)
            vw_sb = cpool.tile([gpc, D], F16)

            with tc.tile_pool(name="gpool", bufs=3) as gpool, \
                 tc.tile_pool(name="rpool", bufs=4) as rpool, \
                 tc.tile_pool(name="pw", bufs=2, space="PSUM") as pwpool, \
                 tc.tile_pool(name="pt", bufs=2, space="PSUM") as ptpool, \
                 tc.tile_pool(name="ps", bufs=1, space="PSUM") as pspool:

                # G buffers memset FIRST on DVE (no input deps) so the
                # first two groups' gathers are not stalled behind the
                # one-hot builds or group-0's R work. Every byte of both
                # buffers must be finite before first use: static (min/max
                # over cores) tile ranges may read slots this core never
                # gathers.
                g_bufs = [gpool.tile([P, GT_MAX, D], F16, tag="G",
                                     name=f"gbuf{i}")
                          for i in range(3)]
                # warm each queue's Q7 pair (one-time ~6us IRAM library
                # load) with a tiny 128-idx gather into scratch before the
                # real stream begins
                for kk in range(min(n_cls, 4)):
                    nc.gpsimd.dma_gather(
                        out_ap=g_bufs[2][:, kk:kk + 1, :],
                        in_ap=hfull_d[0:cls_size[0], :],
                        idxs_ap=warmix_t[:],
                        num_idxs=P,
                        num_idxs_reg=P,
                        elem_size=D,
                        single_packet=False,
                        queue_num=kk % 4,
                    )
                # first-group memset split per class region so gather (0,k)
                # only waits for its own piece
                gt0 = 0
                for kk in range(n_cls):
                    c = capt_gk[kk]
                    if c:
                        nc.vector.memset(g_bufs[0][:, gt0:gt0 + c, :], 0.0)
                    gt0 += c
                nc.vector.memset(g_bufs[1][:], 0.0)
                nc.vector.memset(g_bufs[2][:], 0.0)
                if gt0 < GT_MAX:
                    nc.vector.memset(g_bufs[0][:, gt0:, :], 0.0)

                # batched one-hot builds (one DVE op each):
                # b_all[p, w, g] = (brel[p, w] == g)
                in0 = bass.AP(brel_t[:].tensor, brel_t[:].offset,
                              [list(brel_t[:].ap[0]), [1, NW], [0, gpc]])
                in1 = bass.AP(iotag_t[:].tensor, iotag_t[:].offset,
                              [list(iotag_t[:].ap[0]), [0, NW], [1, gpc]])
                nc.vector.tensor_tensor(out=b_all[:], in0=in0, in1=in1,
                                        op=mybir.AluOpType.is_equal)
                # bT_all[g, j] = (brel_flat[j] == g), staged in 16-window
                # chunks to fit SBUF next to the G buffers
                CHW = 16
                with tc.tile_pool(name="brp", bufs=1) as brpool:
                    for h, j0 in enumerate(range(0, NW, CHW)):
                        j1 = min(j0 + CHW, NW)
                        cols = (j1 - j0) * P
                        brelr_t = brpool.tile([gpc, CHW * P], F16,
                                              tag="brl", name=f"brl{h}")
                        nc.sync.dma_start(
                            out=brelr_t[:, :cols],
                            in_=brelr_d[:, j0 * P:j1 * P])
                        nc.vector.tensor_tensor(
                            out=bT_all[:, j0:j1, :],
                            in0=brelr_t[:, :cols],
                            in1=iotagc_t[:, 0:1].to_broadcast([gpc, cols]),
                            op=mybir.AluOpType.is_equal)

                psum_s = pspool.tile([gpc, D], F32, space="PSUM")

                for g in range(NGRP):
                    w0, w1 = gb[g], gb[g + 1]
                    gbase = rt0[g * n_cls]  # first tile of this group
                    g_t = g_bufs[g % 3]
                    for kk in range(n_cls):
                        capt = capt_gk[g * n_cls + kk]
                        if capt == 0:
                            continue
                        rbase = rt0[g * n_cls + kk]
                        nc.gpsimd.load(
                            gcnt,
                            cnt_t[0:1, g * n_cls + kk:g * n_cls + kk + 1])
                        base = CLASS_SIZE * kk
                        nc.gpsimd.dma_gather(
                            out_ap=g_t[:, rbase - gbase:rbase - gbase + capt,
                                       :],
                            in_ap=hfull_d[base:base + cls_size[kk], :],
                            idxs_ap=idxg[g][:, (rbase - gbase) * 8:
                                            (rbase - gbase + capt) * 8],
                            num_idxs=capt * P,
                            num_idxs_reg=gcnt,
                            elem_size=D,
                            single_packet=False,
                            queue_num=kk % 4,
                        )

                    for wq in range(w0, w1, QW):
                        q = min(QW, w1 - wq)
                        psum_w = pwpool.tile([P, QW, D], F32, space="PSUM",
                                             tag="pw")
                        for j in range(q):
                            w = wq + j
                            runs = []
                            for kk in range(n_cls):
                                lo, hi = (lo_wk[w * n_cls + kk],
                                          hi_wk[w * n_cls + kk])
                                if hi > lo:
                                    runs.append((lo, hi))
                            for ri, (lo, hi) in enumerate(runs):
                                nt = hi - lo
                                r_t = rpool.tile([P, NTMAX, P], F16, tag="R")
                                in0 = drel_t[:, lo:hi].to_broadcast(
                                    [P, nt, P])
                                sl = iota_t[:, 128 * (w % WMOD):
                                            128 * (w % WMOD) + P]
                                in1 = bass.AP(
                                    sl.tensor, sl.offset,
                                    [list(sl.ap[0]), [0, nt], list(sl.ap[1])])
                                nc.vector.tensor_tensor(
                                    out=r_t[:, :nt, :], in0=in0, in1=in1,
                                    op=mybir.AluOpType.is_equal)
                                last_run = ri == len(runs) - 1
                                for t in range(nt):
                                    nc.tensor.matmul(
                                        psum_w[:, j, :],
                                        r_t[:, t, :],
                                        g_t[:, lo - gbase + t, :],
                                        start=(ri == 0 and t == 0),
                                        stop=(last_run and t == nt - 1),
                                        skip_group_check=True)
                        nc.vector.tensor_tensor(
                            out=out_sb[:, wq:wq + q, :],
                            in0=psum_w[:, :q, :],
                            in1=out_sb[:, wq:wq + q, :],
                            op=mybir.AluOpType.add)
                        psum_oT = ptpool.tile([P, QW, P], F16, space="PSUM",
                                              tag="poT")
                        for j in range(q):
                            w = wq + j
                            nc.tensor.matmul(
                                psum_s[:], b_all[:, w, :], out_sb[:, w, :],
                                start=(w == 0), stop=(w == NW - 1),
                                skip_group_check=True)
                            nc.tensor.transpose(psum_oT[:, j, :],
                                                out_sb[:, w, :], ident_t[:])
                        oT_q = rpool.tile([P, QW, P], F16, tag="oT")
                        nc.scalar.copy(oT_q[:, :q, :], psum_oT[:, :q, :])
                        # y1 = out @ W computed here (PE has slack); out_sb
                        # rows are dead after psum_s + transpose, so y1
                        # overwrites them in f16. The tail then only needs
                        # bT @ vW + identity-add of y1.
                        psum_y1 = ptpool.tile([P, QW, D], F32, space="PSUM",
                                              tag="py1")
                        for j in range(q):
                            nc.tensor.matmul(psum_y1[:, j, :],
                                             oT_q[:, j, :], w16_t[:],
                                             start=True, stop=True,
                                             skip_group_check=True)
                        nc.scalar.copy(out_sb[:, wq:wq + q, :],
                                       psum_y1[:, :q, :])

                vmean16 = cpool.tile([gpc, D], F16)
                nc.scalar.mul(vmean16[:], psum_s[:], invc_t[:, 0:1])

            with tc.tile_pool(name="p3", bufs=4) as p3, \
                 tc.tile_pool(name="pp1", bufs=1, space="PSUM") as pp1, \
                 tc.tile_pool(name="pp3", bufs=4, space="PSUM") as pp3:
                psum_vmT = pp1.tile([P, gpc], F16, space="PSUM", tag="pvmT")
                nc.tensor.transpose(psum_vmT[:], vmean16[:],
                                    ident_t[0:gpc, 0:gpc])
                vmT = p3.tile([P, gpc], F16, tag="vmT")
                nc.scalar.copy(vmT[:], psum_vmT[:])
                psum_vw = pp1.tile([gpc, D], F32, space="PSUM", tag="pvw")
                nc.tensor.matmul(psum_vw[:], vmT[:], w16_t[:],
                                 start=True, stop=True)
                nc.scalar.copy(vw_sb[:], psum_vw[:])

                for qi, w0 in enumerate(range(0, NW, QW)):
                    q = min(QW, NW - w0)
                    psum_yq = pp3.tile([P, QW, D], F32, space="PSUM",
                                       tag="py")
                    for j in range(q):
                        w = w0 + j
                        nc.tensor.matmul(psum_yq[:, j, :], bT_all[:, w, :],
                                         vw_sb[:], start=True, stop=True,
                                         skip_group_check=True)
                    # += y1 (stored in out_sb) on DVE (idle at the tail),
                    # relu on Scalar, DMA alternating both HWDGE queues
                    ys_t = p3.tile([P, QW, D], F32, tag="YS")
                    nc.vector.tensor_tensor(
                        out=ys_t[:, :q, :], in0=psum_yq[:, :q, :],
                        in1=out_sb[:, w0:w0 + q, :],
                        op=mybir.AluOpType.add)
                    yq_t = p3.tile([P, QW, D], F32, tag="Y")
                    nc.scalar.activation(yq_t[:, :q, :], ys_t[:, :q, :],
                                         mybir.ActivationFunctionType.Relu)
                    out_ap = bass.AP(
                        y_d[:].tensor, w0 * P * D,
                        [[D, P], [P * D, q], [1, D]])
                    eng = nc.sync if qi % 2 == 0 else nc.scalar
                    eng.dma_start(out=out_ap, in_=yq_t[:, :q, :])
    _finish_compile(nc)
    return nc


def _finish_compile(nc):
    nc.compile()
    # compile()'s tail passes (library-load insertion for the custom DMA
    # instructions) can reintroduce >1 sync wait per instruction, which the
    # TRN2 ISA rejects. Re-split and re-codegen.
    import bass_rust
    bass_rust.generate_event_semaphores(nc)
    nc.codegen_inst_isa_subclasses()


_BUILD_CACHE = {}


def _build_cached(params):
    key = tuple(sorted((k, v) for k, v in params.items()))
    if key not in _BUILD_CACHE:
        _BUILD_CACHE[key] = _build(params)
    return _BUILD_CACHE[key]


def _run(H, edge_index, batch, W, n_graphs, trace=False):
    H = np.asarray(H)
    params, in_maps, n_c, core_start = _prep(H, edge_index, batch, n_graphs)
    consts = _consts(params, np.asarray(W))
    hfull16 = np.ascontiguousarray(H.astype(np.float16))
    for m in in_maps:
        m["hfull16"] = hfull16
        m.update(consts)
    nc = _build_cached(params)
    res = run_bass_kernel_spmd(nc, in_maps, list(range(N_CORES)), trace=trace)
    N = H.shape[0]
    y = np.empty((N, D), dtype=np.float32)
    for c in range(N_CORES):
        y[core_start[c]:core_start[c] + n_c[c]] = \
            res.results[c]["y"][:n_c[c]]
    return y, res


def kernel(H, edge_index, batch, W):
    y, _ = _run(H, edge_index, batch, W, n_graphs=256,
                trace=bool(os.environ.get("GCN_TRACE")))
    return y
